# revision 1
# baseline (speedup 1.0000x reference)
"""Trainium2 Bass kernel for nn_CombinedLoss (chamfer + repulsion + PCA-normal
consistency) on point clouds [8, 2048, 3].

Sharding: data-parallel over batch B=8 across 8 NeuronCores (1 sample/core).
Per core the device computes the O(N^2) work:
  - negated squared-distance matrices -Dpg, -Dpp, -Dgg via PE matmuls
    (augmented K=5 contraction folds in the |p|^2/|g|^2 terms)
  - chamfer row/col max reductions (of -D)
  - repulsion moment accumulations s1 = sum relu(r^2 - d2), s2 = sum relu^2
  - 16-NN radius per row via segmented-max tree + max8/match_replace/max8
  - neighbor mask (bf16) -> DMA-transposed -> PE mask @ features matmul
    giving second moments / mean / count per point (the 3x3 PCA covariances)
Host combines the 8 cores' small outputs: chamfer means, repulsion tail
(quadratic moment inversion + sqrt), covariance assembly, and the smallest
eigenvector of each 3x3 cov with LAPACK ssyevd's exact sign convention
(vectorized fp32 replication of ssytd2+ssteqr+sorm2r, validated 100% against
jax CPU eigh), then the weighted loss.
"""

import numpy as np

try:
    import ml_dtypes

    BF16 = ml_dtypes.bfloat16
except Exception:  # pragma: no cover
    BF16 = None

B, N, DIM = 8, 2048, 3
K_REP = 4
REP_THRESH = np.float32(0.02)
K_NORM = 16
CD_W, REP_W, NORM_W = 1.0, 0.1, 0.01
NB = N // 128  # 16 row blocks
NEG_BIG = np.float32(-1e30)

# ============================================================================
# LAPACK ssyevd 3x3 sign-convention replication (fp32, vectorized, masked).
# Validated to match jax/scipy CPU eigh signs 20000/20000.
# ============================================================================
F = np.float32
EPS_L = F(2.0) ** F(-24)
EPS2_L = F(EPS_L * EPS_L)
SAFMIN_L = F(1.1754943508222875e-38)
ONE = F(1.0)
TWO = F(2.0)
HALF = F(0.5)
ZERO = F(0.0)


def _fsign(a, b):
    return np.where(b >= 0, np.abs(a), -np.abs(a)).astype(np.float32)


def _slapy2(x, y):
    ax = np.abs(x); ay = np.abs(y)
    w = np.maximum(ax, ay)
    z = np.minimum(ax, ay)
    ratio = z / np.where(w == 0, ONE, w)
    res = w * np.sqrt(ONE + ratio * ratio)
    return np.where(z == 0, w, res).astype(np.float32)


def _slartg(f, g):
    # LAPACK 3.10+ slartg, fast path
    d = np.sqrt(f * f + g * g).astype(np.float32)
    f1 = np.abs(f)
    cs = (f1 / d).astype(np.float32)
    r = _fsign(d, f)
    sn = (g / r).astype(np.float32)
    cs = np.where(g == 0, ONE, cs)
    sn = np.where(g == 0, ZERO, sn)
    r = np.where(g == 0, f, r)
    f0 = (f == 0) & (g != 0)
    cs = np.where(f0, ZERO, cs)
    sn = np.where(f0, _fsign(np.ones_like(g), g), sn)
    r = np.where(f0, np.abs(g), r)
    return cs, sn, r


def _slaev2(a, b, c):
    sm = a + c
    df = a - c
    adf = np.abs(df)
    tb = b + b
    ab_ = np.abs(tb)
    acmx = np.where(np.abs(a) > np.abs(c), a, c)
    acmn = np.where(np.abs(a) > np.abs(c), c, a)
    r_adf = adf * np.sqrt(ONE + (ab_ / np.where(adf == 0, ONE, adf)) ** 2)
    r_ab = ab_ * np.sqrt(ONE + (adf / np.where(ab_ == 0, ONE, ab_)) ** 2)
    r_eq = ab_ * np.sqrt(TWO)
    rt = np.where(adf > ab_, r_adf, np.where(adf < ab_, r_ab, r_eq)).astype(np.float32)
    sm_neg = sm < 0
    sm_pos = sm > 0
    rt1 = np.where(sm_neg, HALF * (sm - rt), np.where(sm_pos, HALF * (sm + rt), HALF * rt)).astype(np.float32)
    safe_rt1 = np.where(rt1 == 0, ONE, rt1)
    rt2_gen = ((acmx / safe_rt1) * acmn - (b / safe_rt1) * b).astype(np.float32)
    rt2 = np.where(sm_neg | sm_pos, rt2_gen, (-HALF * rt).astype(np.float32)).astype(np.float32)
    sgn1 = np.where(sm_neg, -ONE, ONE).astype(np.float32)
    df_ge = df >= 0
    cs = np.where(df_ge, df + rt, df - rt).astype(np.float32)
    sgn2 = np.where(df_ge, ONE, -ONE).astype(np.float32)
    acs = np.abs(cs)
    ct = (-tb / np.where(cs == 0, ONE, cs)).astype(np.float32)
    sn1_a = (ONE / np.sqrt(ONE + ct * ct)).astype(np.float32)
    cs1_a = (ct * sn1_a).astype(np.float32)
    ab_zero = ab_ == 0
    tn = (-cs / np.where(ab_zero, ONE, tb)).astype(np.float32)
    cs1_b = (ONE / np.sqrt(ONE + tn * tn)).astype(np.float32)
    sn1_b = (tn * cs1_b).astype(np.float32)
    cs1_b = np.where(ab_zero, ONE, cs1_b)
    sn1_b = np.where(ab_zero, ZERO, sn1_b)
    use_a = acs > ab_
    cs1 = np.where(use_a, cs1_a, cs1_b).astype(np.float32)
    sn1 = np.where(use_a, sn1_a, sn1_b).astype(np.float32)
    flip = sgn1 == sgn2
    cs1_f = np.where(flip, -sn1, cs1).astype(np.float32)
    sn1_f = np.where(flip, cs1, sn1).astype(np.float32)
    return rt1, rt2, cs1_f, sn1_f


def eigh3_smallest_lapack(A):
    """A: [M,3,3] fp32 symmetric -> [M,3] smallest-eigval eigenvector with
    LAPACK ssyevd (3.10+) sign convention."""
    with np.errstate(all="ignore"):
        return _eigh3_smallest_lapack(A)


def _eigh3_smallest_lapack(A):
    A = np.asarray(A, dtype=np.float32)
    M = A.shape[0]
    a00 = A[:, 0, 0].copy(); a10 = A[:, 1, 0].copy(); a20 = A[:, 2, 0].copy()
    a11 = A[:, 1, 1].copy(); a21 = A[:, 2, 1].copy(); a22 = A[:, 2, 2].copy()
    # ssytd2 lower
    xnorm = np.abs(a20)
    alpha = a10
    beta = -_fsign(_slapy2(alpha, xnorm), alpha)
    refl = xnorm != 0
    safe_beta = np.where(refl, beta, ONE)
    tau1 = np.where(refl, (beta - alpha) / safe_beta, ZERO).astype(np.float32)
    denom = np.where(refl, alpha - beta, ONE)
    v2 = np.where(refl, a20 / denom, ZERO).astype(np.float32)
    w1 = (tau1 * a11 + tau1 * (a21 * v2)).astype(np.float32)
    w2 = (tau1 * a21 + (tau1 * v2) * a22).astype(np.float32)
    alp = (-HALF * tau1 * (w1 + w2 * v2)).astype(np.float32)
    w1 = (w1 + alp).astype(np.float32)
    w2 = (w2 + alp * v2).astype(np.float32)
    d = [a00,
         np.where(refl, (a11 - (w1 + w1)).astype(np.float32), a11),
         np.where(refl, (a22 - ((v2 * w2) + (v2 * w2))).astype(np.float32), a22)]
    e = [np.where(refl, beta, a10),
         np.where(refl, (a21 - (v2 * w1 + w2)).astype(np.float32), a21)]
    Z = np.zeros((M, 3, 3), dtype=np.float32)
    Z[:, 0, 0] = 1; Z[:, 1, 1] = 1; Z[:, 2, 2] = 1

    thr0 = ((np.sqrt(np.abs(d[0])) * np.sqrt(np.abs(d[1]))) * EPS_L).astype(np.float32)
    s0 = np.abs(e[0]) <= thr0
    thr1 = ((np.sqrt(np.abs(d[1])) * np.sqrt(np.abs(d[2]))) * EPS_L).astype(np.float32)
    s1m = np.abs(e[1]) <= thr1
    e[0] = np.where(s0, ZERO, e[0])
    e[1] = np.where(s1m, ZERO, e[1])

    def apply_rot(ca, cb, c, s, mask):
        temp = Z[:, :, cb].copy()
        zb = (c[:, None] * temp - s[:, None] * Z[:, :, ca]).astype(np.float32)
        za = (s[:, None] * temp + c[:, None] * Z[:, :, ca]).astype(np.float32)
        m = mask[:, None]
        Z[:, :, cb] = np.where(m, zb, Z[:, :, cb])
        Z[:, :, ca] = np.where(m, za, Z[:, :, ca])

    def proc_2x2(da, eab, db, ca, cb, mask):
        tst = (eab * eab).astype(np.float32)
        thr = ((EPS2_L * np.abs(da)) * np.abs(db) + SAFMIN_L).astype(np.float32)
        defl = tst <= thr
        act = mask & ~defl
        rt1, rt2, c, s = _slaev2(da, eab, db)
        apply_rot(ca, cb, c, s, act)
        da_n = np.where(act, rt1, da)
        db_n = np.where(act, rt2, db)
        e_n = np.where(mask, ZERO, eab)
        return da_n, e_n, db_n

    m_tf = s0 & ~s1m
    d[1], e[1], d[2] = proc_2x2(d[1], e[1], d[2], 1, 2, m_tf)
    m_ft = ~s0 & s1m
    d[0], e[0], d[1] = proc_2x2(d[0], e[0], d[1], 0, 1, m_ft)

    m_ff = ~s0 & ~s1m
    use_qr = np.abs(d[2]) < np.abs(d[0])
    m_ql = m_ff & ~use_qr
    m_qr = m_ff & use_qr

    def ql_step(l, active):
        l_new = l.copy()
        at0 = active & (l == 0)
        if at0.any():
            tst0 = (e[0] * e[0]).astype(np.float32)
            thr0_ = ((EPS2_L * np.abs(d[0])) * np.abs(d[1]) + SAFMIN_L).astype(np.float32)
            m0s = tst0 <= thr0_
            tst1 = (e[1] * e[1]).astype(np.float32)
            thr1_ = ((EPS2_L * np.abs(d[1])) * np.abs(d[2]) + SAFMIN_L).astype(np.float32)
            m1s = tst1 <= thr1_
            conv0 = at0 & m0s
            e[0] = np.where(conv0, ZERO, e[0])
            l_new = np.where(conv0, 1, l_new)
            blk2 = at0 & ~m0s & m1s
            e[1] = np.where(blk2, ZERO, e[1])
            if blk2.any():
                rt1, rt2, c, s = _slaev2(d[0], e[0], d[1])
                apply_rot(0, 1, c, s, blk2)
                d[0] = np.where(blk2, rt1, d[0])
                d[1] = np.where(blk2, rt2, d[1])
                e[0] = np.where(blk2, ZERO, e[0])
            l_new = np.where(blk2, 2, l_new)
            sweep = at0 & ~m0s & ~m1s
            if sweep.any():
                P = d[0]
                G = ((d[1] - P) / (TWO * np.where(sweep, e[0], ONE))).astype(np.float32)
                R = _slapy2(G, np.ones_like(G))
                G = (d[2] - P + (e[0] / (G + _fsign(R, G)))).astype(np.float32)
                Fv = e[1].astype(np.float32)
                Bv = e[1].astype(np.float32)
                C, S, R = _slartg(G, Fv)
                G2 = d[2]
                R = ((d[1] - G2) * S + (TWO * C) * Bv).astype(np.float32)
                Pv = (S * R).astype(np.float32)
                d2n = (G2 + Pv).astype(np.float32)
                G = (C * R - Bv).astype(np.float32)
                c1 = C.copy(); s1_ = (-S).astype(np.float32)
                Fv = (S * e[0]).astype(np.float32)
                Bv = (C * e[0]).astype(np.float32)
                C, S, R = _slartg(G, Fv)
                e1n = R
                G2 = (d[1] - Pv).astype(np.float32)
                R = ((d[0] - G2) * S + (TWO * C) * Bv).astype(np.float32)
                Pv2 = (S * R).astype(np.float32)
                d1n = (G2 + Pv2).astype(np.float32)
                G = (C * R - Bv).astype(np.float32)
                c0 = C.copy(); s0_ = (-S).astype(np.float32)
                apply_rot(1, 2, c1, s1_, sweep)
                apply_rot(0, 1, c0, s0_, sweep)
                d[2] = np.where(sweep, d2n, d[2])
                d[1] = np.where(sweep, d1n, d[1])
                d[0] = np.where(sweep, (d[0] - Pv2).astype(np.float32), d[0])
                e[1] = np.where(sweep, e1n, e[1])
                e[0] = np.where(sweep, G, e[0])
        at1 = active & (l == 1) & (l_new == l)
        if at1.any():
            tst1 = (e[1] * e[1]).astype(np.float32)
            thr1_ = ((EPS2_L * np.abs(d[1])) * np.abs(d[2]) + SAFMIN_L).astype(np.float32)
            m1s = tst1 <= thr1_
            conv1 = at1 & m1s
            e[1] = np.where(conv1, ZERO, e[1])
            l_new = np.where(conv1, 2, l_new)
            blk2 = at1 & ~m1s
            if blk2.any():
                rt1, rt2, c, s = _slaev2(d[1], e[1], d[2])
                apply_rot(1, 2, c, s, blk2)
                d[1] = np.where(blk2, rt1, d[1])
                d[2] = np.where(blk2, rt2, d[2])
                e[1] = np.where(blk2, ZERO, e[1])
            l_new = np.where(blk2, 3, l_new)
        at2 = active & (l == 2) & (l_new == l)
        l_new = np.where(at2, 3, l_new)
        return l_new

    def qr_step(l, active):
        l_new = l.copy()
        at2 = active & (l == 2)
        if at2.any():
            tst1 = (e[1] * e[1]).astype(np.float32)
            thr1_ = ((EPS2_L * np.abs(d[2])) * np.abs(d[1]) + SAFMIN_L).astype(np.float32)
            m2s = tst1 <= thr1_
            tst0 = (e[0] * e[0]).astype(np.float32)
            thr0_ = ((EPS2_L * np.abs(d[1])) * np.abs(d[0]) + SAFMIN_L).astype(np.float32)
            m1s = tst0 <= thr0_
            conv2 = at2 & m2s
            e[1] = np.where(conv2, ZERO, e[1])
            l_new = np.where(conv2, 1, l_new)
            blk2 = at2 & ~m2s & m1s
            e[0] = np.where(blk2, ZERO, e[0])
            if blk2.any():
                rt1, rt2, c, s = _slaev2(d[1], e[1], d[2])
                apply_rot(1, 2, c, s, blk2)
                d[1] = np.where(blk2, rt1, d[1])
                d[2] = np.where(blk2, rt2, d[2])
                e[1] = np.where(blk2, ZERO, e[1])
            l_new = np.where(blk2, 0, l_new)
            sweep = at2 & ~m2s & ~m1s
            if sweep.any():
                P = d[2]
                G = ((d[1] - P) / (TWO * np.where(sweep, e[1], ONE))).astype(np.float32)
                R = _slapy2(G, np.ones_like(G))
                G = (d[0] - P + (e[1] / (G + _fsign(R, G)))).astype(np.float32)
                Fv = e[0].astype(np.float32)
                Bv = e[0].astype(np.float32)
                C, S, R = _slartg(G, Fv)
                G2 = d[0]
                R = ((d[1] - G2) * S + (TWO * C) * Bv).astype(np.float32)
                Pv = (S * R).astype(np.float32)
                d0n = (G2 + Pv).astype(np.float32)
                G = (C * R - Bv).astype(np.float32)
                c0 = C.copy(); s0_ = S.copy()
                Fv = (S * e[1]).astype(np.float32)
                Bv = (C * e[1]).astype(np.float32)
                C, S, R = _slartg(G, Fv)
                e0n = R
                G2 = (d[1] - Pv).astype(np.float32)
                R = ((d[2] - G2) * S + (TWO * C) * Bv).astype(np.float32)
                Pv2 = (S * R).astype(np.float32)
                d1n = (G2 + Pv2).astype(np.float32)
                G = (C * R - Bv).astype(np.float32)
                c1 = C.copy(); s1_ = S.copy()
                apply_rot(0, 1, c0, s0_, sweep)
                apply_rot(1, 2, c1, s1_, sweep)
                d[0] = np.where(sweep, d0n, d[0])
                d[1] = np.where(sweep, d1n, d[1])
                d[2] = np.where(sweep, (d[2] - Pv2).astype(np.float32), d[2])
                e[0] = np.where(sweep, e0n, e[0])
                e[1] = np.where(sweep, G, e[1])
        at1 = active & (l == 1) & (l_new == l)
        if at1.any():
            tst0 = (e[0] * e[0]).astype(np.float32)
            thr0_ = ((EPS2_L * np.abs(d[1])) * np.abs(d[0]) + SAFMIN_L).astype(np.float32)
            ms = tst0 <= thr0_
            conv = at1 & ms
            e[0] = np.where(conv, ZERO, e[0])
            l_new = np.where(conv, 0, l_new)
            blk2 = at1 & ~ms
            if blk2.any():
                rt1, rt2, c, s = _slaev2(d[0], e[0], d[1])
                apply_rot(0, 1, c, s, blk2)
                d[0] = np.where(blk2, rt1, d[0])
                d[1] = np.where(blk2, rt2, d[1])
                e[0] = np.where(blk2, ZERO, e[0])
            l_new = np.where(blk2, -1, l_new)
        at0 = active & (l == 0) & (l_new == l)
        l_new = np.where(at0, -1, l_new)
        return l_new

    l_ql = np.zeros(M, dtype=np.int32)
    l_qr = np.full(M, 2, dtype=np.int32)
    for _ in range(40):
        act_ql = m_ql & (l_ql < 3)
        if act_ql.any():
            l_ql = ql_step(l_ql, act_ql)
        act_qr = m_qr & (l_qr > -1)
        if act_qr.any():
            l_qr = qr_step(l_qr, act_qr)
        if not ((m_ql & (l_ql < 3)).any() or (m_qr & (l_qr > -1)).any()):
            break

    D = np.stack(d, axis=1)

    def sort_step(D, i):
        K = np.full(M, i, dtype=np.int32)
        P = D[:, i].copy()
        for j in range(i + 1, 3):
            upd = D[:, j] < P
            K = np.where(upd, j, K)
            P = np.where(upd, D[:, j], P)
        for k in range(i + 1, 3):
            m = K == k
            if m.any():
                D[:, k] = np.where(m, D[:, i], D[:, k])
                D[:, i] = np.where(m, P, D[:, i])
                zi = Z[:, :, i].copy(); zk = Z[:, :, k].copy()
                mm = m[:, None]
                Z[:, :, i] = np.where(mm, zk, Z[:, :, i])
                Z[:, :, k] = np.where(mm, zi, Z[:, :, k])
        return D

    D = sort_step(D, 0)
    D = sort_step(D, 1)

    w = (Z[:, 1, :] + v2[:, None] * Z[:, 2, :]).astype(np.float32)
    z1n = (Z[:, 1, :] - tau1[:, None] * w).astype(np.float32)
    z2n = (Z[:, 2, :] - (tau1[:, None] * v2[:, None]) * w).astype(np.float32)
    Z[:, 1, :] = np.where(refl[:, None], z1n, Z[:, 1, :])
    Z[:, 2, :] = np.where(refl[:, None], z2n, Z[:, 2, :])
    return Z[:, :, 0]


# ============================================================================
# Host-side input prep (per core / sample)
# ============================================================================

def _prep_core_inputs(p, g):
    """p, g: [N, 3] fp32. Build the per-core device input dict."""
    f32 = np.float32
    xx = (p * p).sum(-1).astype(f32)   # [N]
    yy = (g * g).sum(-1).astype(f32)
    ones = np.ones(N, dtype=f32)

    def _hilo(v):
        hi = v.astype(BF16)
        lo = (v - hi.astype(f32)).astype(BF16)
        return hi, lo

    zpad = np.zeros((128 - 15, N), dtype=BF16)

    def lhs(pts, nn):
        # [128, N] bf16: hi(5), hi(5), lo(5) of rows [2x, 2y, 2z, nn, 1], zero pad
        v = np.stack([2 * pts[:, 0], 2 * pts[:, 1], 2 * pts[:, 2], nn, ones], 0).astype(f32)
        hi, lo = _hilo(v)
        return np.concatenate([hi, hi, lo, zpad], 0)

    def rhs(pts, nn):
        # [128, N] bf16: hi(5), lo(5), hi(5) of rows [x, y, z, -1, -nn], zero pad
        v = np.stack([pts[:, 0], pts[:, 1], pts[:, 2], -ones, -nn], 0).astype(f32)
        hi, lo = _hilo(v)
        return np.concatenate([hi, lo, hi, zpad], 0)

    def feats(pts):
        # F rows: [x2, xy, xz, y2, yz, z2, x, y, z, 1] with centered coords
        c = (pts - f32(0.5)).astype(f32)
        x, y, z = c[:, 0], c[:, 1], c[:, 2]
        Fm = np.stack([x * x, x * y, x * z, y * y, y * z, z * z, x, y, z, ones], 0).astype(f32)  # [10, N]
        return Fm

    fp = feats(p)
    fg = feats(g)
    # bf16 hi/lo split, layout [128, NB, 20] (chunk kb -> [:, kb, 0:10]=hi, [:,kb,10:20]=lo)
    def ft_split(Fm):
        hi = Fm.astype(BF16)
        lo = (Fm - hi.astype(f32)).astype(BF16)
        # [10, N] -> [N, 10] -> [NB, 128, 10] -> [128, NB, 10]
        hiT = np.ascontiguousarray(hi.T.reshape(NB, 128, 10).transpose(1, 0, 2))
        loT = np.ascontiguousarray(lo.T.reshape(NB, 128, 10).transpose(1, 0, 2))
        return np.concatenate([hiT, loT], axis=2)  # [128, NB, 20]

    negdiag = np.zeros((128, 128), dtype=BF16)
    np.fill_diagonal(negdiag, BF16(NEG_BIG))
    ident = np.zeros((128, 128), dtype=BF16)
    np.fill_diagonal(ident, BF16(1.0))

    return {
        "ident": ident,
        "lhs_p": lhs(p, xx), "rhs_p": rhs(p, xx),
        "lhs_g": lhs(g, yy), "rhs_g": rhs(g, yy),
        "ft_p": ft_split(fp), "ft_g": ft_split(fg),
        "frow_p": fp, "frow_g": fg,
        "negdiag": negdiag,
    }


# ============================================================================
# Bass device kernel builder
# ============================================================================

def _build_nc():
    import concourse.bass as bass
    import concourse.mybir as mybir
    from concourse.tile import TileContext

    f32 = mybir.dt.float32
    f32r = mybir.dt.float32r
    bf16 = mybir.dt.bfloat16
    Alu = mybir.AluOpType
    Act = mybir.ActivationFunctionType
    Axis = mybir.AxisListType

    nc = bass.Bass()

    # ---- DRAM io ----
    lhs_p = nc.dram_tensor("lhs_p", [128, N], bf16, kind="ExternalInput")
    rhs_p = nc.dram_tensor("rhs_p", [128, N], bf16, kind="ExternalInput")
    lhs_g = nc.dram_tensor("lhs_g", [128, N], bf16, kind="ExternalInput")
    rhs_g = nc.dram_tensor("rhs_g", [128, N], bf16, kind="ExternalInput")
    ft_p = nc.dram_tensor("ft_p", [128, NB, 20], bf16, kind="ExternalInput")
    ft_g = nc.dram_tensor("ft_g", [128, NB, 20], bf16, kind="ExternalInput")
    frow_p = nc.dram_tensor("frow_p", [10, N], f32, kind="ExternalInput")
    frow_g = nc.dram_tensor("frow_g", [10, N], f32, kind="ExternalInput")
    negdiag = nc.dram_tensor("negdiag", [128, 128], bf16, kind="ExternalInput")
    ident = nc.dram_tensor("ident", [128, 128], bf16, kind="ExternalInput")

    rowmax_pg = nc.dram_tensor("rowmax_pg", [128, 2 * NB], f32, kind="ExternalOutput")
    colmax_pg = nc.dram_tensor("colmax_pg", [1, N], f32, kind="ExternalOutput")
    s1_out = nc.dram_tensor("s1_out", [128, NB], f32, kind="ExternalOutput")
    s2_out = nc.dram_tensor("s2_out", [128, NB], f32, kind="ExternalOutput")
    cov_p = nc.dram_tensor("cov_p", [10, N], f32, kind="ExternalOutput")
    cov_g = nc.dram_tensor("cov_g", [10, N], f32, kind="ExternalOutput")

    with TileContext(nc) as tc:
        import contextlib
        ctx = contextlib.ExitStack()
        with ctx:
            aug = ctx.enter_context(tc.tile_pool(name="aug", bufs=1))
            small = ctx.enter_context(tc.tile_pool(name="small", bufs=1))
            ndmp = ctx.enter_context(tc.tile_pool(name="ndmp", bufs=1))
            wtp = ctx.enter_context(tc.tile_pool(name="wtp", bufs=1))
            wrowp = ctx.enter_context(tc.tile_pool(name="wrowp", bufs=1))
            scrp = ctx.enter_context(tc.tile_pool(name="scrp", bufs=1))
            psd = ctx.enter_context(tc.tile_pool(name="psd", bufs=2, space="PSUM"))
            psc = ctx.enter_context(tc.tile_pool(name="psc", bufs=1, space="PSUM"))

            # ---- load inputs to SBUF ----
            t_lhs_p = aug.tile([128, N], bf16, tag="lhsA")
            t_rhs_p = aug.tile([128, N], bf16, tag="rhsp")
            t_rhs_g = aug.tile([128, N], bf16, tag="rhsg")
            t_ft_p = aug.tile([128, NB, 20], bf16, tag="ftp")
            t_ft_g = aug.tile([128, NB, 20], bf16, tag="ftg")
            t_negdiag = aug.tile([128, 128], bf16, tag="ndg")
            t_ident = aug.tile([128, 128], bf16, tag="ident")
            for dst, srct in [(t_lhs_p, lhs_p), (t_rhs_p, rhs_p),
                              (t_rhs_g, rhs_g), (t_ft_p, ft_p), (t_ft_g, ft_g),
                              (t_negdiag, negdiag), (t_ident, ident)]:
                nc.sync.dma_start(dst[:], srct[:])

            # ---- small result tiles ----
            t_ones = small.tile([128, 128], bf16, tag="ones")
            nc.vector.memset(t_ones[:], 1.0)
            t_bias4 = small.tile([128, 1], f32, tag="bias4")
            t_bias0 = small.tile([128, 1], f32, tag="bias0")
            nc.vector.memset(t_bias4[:], float(REP_THRESH * REP_THRESH))
            nc.vector.memset(t_bias0[:], 0.0)
            t_rowmax = small.tile([128, 2 * NB], f32, tag="rowmax")
            t_colacc = small.tile([128, N], f32, tag="bigA")
            t_s1 = small.tile([128, NB], f32, tag="s1")
            t_s2 = small.tile([128, NB], f32, tag="s2")
            t_colred = small.tile([128, N], f32, tag="bigB")

            def build_half(lhsT, rhsT, b, h, ps):
                # -D row block b, column half h: out [128, 1024] psum;
                # K=128 bf16 (hi/lo packed, zero padded); N=512 per MM
                for j in range(2):
                    nc.tensor.matmul(
                        ps[:, j * 512:(j + 1) * 512],
                        lhsT[:, b * 128:(b + 1) * 128],
                        rhsT[:, h * 1024 + j * 512:h * 1024 + (j + 1) * 512],
                        start=True, stop=True,
                    )

            # ================= phase 1: chamfer on -Dpg =================
            for b in range(NB):
                for h in range(2):
                    ps = psd.tile([128, 1024], f32, tag="dps")
                    build_half(t_lhs_p, t_rhs_g, b, h, ps)
                    nc.vector.tensor_reduce(t_rowmax[:, 2 * b + h:2 * b + h + 1],
                                            ps[:], Axis.X, Alu.max)
                    cslice = slice(h * 1024, (h + 1) * 1024)
                    if b == 0:
                        nc.vector.tensor_copy(t_colacc[:, cslice], ps[:])
                    else:
                        nc.vector.tensor_tensor(t_colacc[:, cslice],
                                                t_colacc[:, cslice], ps[:], Alu.max)
            # partition-tree max: DMA the upper half down 64->0 partitions,
            # TT max, repeat (DVE cannot cross partitions; DMA can)
            for h in [64, 32, 16, 8, 4, 2, 1]:
                nc.sync.dma_start(t_colred[0:h, :], t_colacc[h:2 * h, :])
                nc.vector.tensor_tensor(t_colacc[0:h, :], t_colacc[0:h, :],
                                        t_colred[0:h, :], Alu.max)
            nc.sync.dma_start(rowmax_pg[:], t_rowmax[:])
            nc.sync.dma_start(colmax_pg[:], t_colacc[0:1, :])

            # ================= phases 2-4 for pp and gg =================
            def normals_phase(lhsT, rhsT, t_ft, frow_dram, cov_out, do_rep):
                t_frow = small.tile([10, N], f32, tag="bigB")
                nc.sync.dma_start(t_frow[:], frow_dram[:])
                ndm = [ndmp.tile([128, N], bf16, tag=f"ndm{i}", name=f"ndm{i}")
                       for i in range(NB)]
                # build + bf16 copy + diag mask (+ rep)
                for b in range(NB):
                    for h in range(2):
                        ps = psd.tile([128, 1024], f32, tag="dps")
                        build_half(lhsT, rhsT, b, h, ps)
                        nc.scalar.activation(ndm[b][:, h * 1024:(h + 1) * 1024],
                                             ps[:], mybir.ActivationFunctionType.Copy)
                    nc.vector.tensor_tensor(
                        ndm[b][:, b * 128:(b + 1) * 128],
                        ndm[b][:, b * 128:(b + 1) * 128],
                        t_negdiag[:], Alu.add)
                    if do_rep:
                        scr = scrp.tile([128, N], bf16, tag="repscr")
                        scr2 = scrp.tile([128, N], bf16, tag="repscr2")
                        nc.scalar.activation(scr[:], ndm[b][:], Act.Relu,
                                             bias=t_bias4[:],
                                             accum_out=t_s1[:, b:b + 1])
                        nc.scalar.activation(scr2[:], scr[:], Act.Square,
                                             bias=t_bias0[:],
                                             accum_out=t_s2[:, b:b + 1])
                # selection: tree max -> A [128, 512] -> max8 chain -> tau
                t_tau = small.tile([128, NB], f32, tag="tau")
                for b in range(NB):
                    t1 = scrp.tile([128, 1024], bf16, tag="tree1")
                    A = scrp.tile([128, 512], bf16, tag="treeA")
                    A2 = scrp.tile([128, 512], bf16, tag="treeA2")
                    m8a = scrp.tile([128, 8], bf16, tag="m8a")
                    m8b = scrp.tile([128, 8], bf16, tag="m8b")
                    nc.vector.tensor_tensor(t1[:], ndm[b][:, 0:1024], ndm[b][:, 1024:2048], Alu.max)
                    nc.vector.tensor_tensor(A[:], t1[:, 0:512], t1[:, 512:1024], Alu.max)
                    nc.vector.max(m8a[:], A[:])
                    nc.vector.match_replace(A2[:], m8a[:], A[:], float(NEG_BIG))
                    nc.vector.max(m8b[:], A2[:])
                    nc.vector.tensor_copy(t_tau[:, b:b + 1], m8b[:, 6:7])
                # tau broadcast: gather per-row -tau into a [1, N] row (bf16),
                # then PE ones-matmul broadcasts it across partitions; the
                # transposed mask is then a direct compare on the SYMMETRIC
                # ndm row blocks: wt[j, i] = (ndm[j, i] >= taubc[j, i]=tau_i)
                t_taub = wrowp.tile([128, 128], bf16, tag="taub")
                nc.vector.memset(t_taub[:], 0.0)
                nc.vector.tensor_copy(t_taub[:, 0:NB], t_tau[:])
                ps_tt = psd.tile([128, 128], bf16, tag="dps")
                nc.tensor.transpose(ps_tt[:], t_taub[:], t_ident[:])
                t_tt = wrowp.tile([NB, 128], bf16, tag="tts")
                nc.scalar.activation(t_tt[:], ps_tt[0:NB, :],
                                     mybir.ActivationFunctionType.Copy)
                t_tauT = wrowp.tile([128, N], bf16, tag="tauT")
                nc.vector.memset(t_tauT[:], 0.0)
                nc.sync.dma_start(t_tauT[0:1, :], t_tt[:])
                t_taubc = wrowp.tile([128, N], bf16, tag="taubc")
                for h in range(2):
                    ps_tau = psd.tile([128, 1024], f32, tag="dps")
                    for bb in range(8):
                        c0 = h * 1024 + bb * 128
                        nc.tensor.matmul(ps_tau[:, bb * 128:(bb + 1) * 128],
                                         t_ones[:],
                                         t_tauT[:, c0:c0 + 128],
                                         start=True, stop=True)
                    nc.scalar.activation(t_taubc[:, h * 1024:(h + 1) * 1024],
                                         ps_tau[:],
                                         mybir.ActivationFunctionType.Copy)
                wt = [wtp.tile([128, N], bf16, tag=f"wt{i}", name=f"wt{i}")
                      for i in range(NB)]
                for jb in range(NB):
                    nc.vector.tensor_tensor(wt[jb][:], ndm[jb][:],
                                            t_taubc[:], Alu.is_ge)
                # cov matmul: psum [10, N] accumulate over kb chunks, hi+lo
                cps = psc.tile([10, N], f32, tag="cps")
                for j in range(4):
                    cols = slice(j * 512, (j + 1) * 512)
                    first = True
                    for kb in range(NB):
                        for half in range(2):
                            nc.tensor.matmul(
                                cps[:, cols],
                                t_ft[:, kb, half * 10:(half + 1) * 10],
                                wt[kb][:, cols],
                                start=first, stop=(kb == NB - 1 and half == 1))
                            first = False
                # self add + out
                covsb = small.tile([10, N], f32, tag="bigA")
                nc.vector.tensor_tensor(covsb[:], cps[:], t_frow[:], Alu.add)
                nc.sync.dma_start(cov_out[:], covsb[:])

            normals_phase(t_lhs_p, t_rhs_p, t_ft_p, frow_p, cov_p, do_rep=True)
            nc.sync.dma_start(s1_out[:], t_s1[:])
            nc.sync.dma_start(s2_out[:], t_s2[:])
            t_lhs_g = aug.tile([128, N], bf16, tag="lhsA")
            nc.sync.dma_start(t_lhs_g[:], lhs_g[:])
            normals_phase(t_lhs_g, t_rhs_g, t_ft_g, frow_g, cov_g, do_rep=False)

    _split_excess_waits(nc, mybir)
    return nc




def _split_excess_waits(nc, mybir, max_w=1, max_u=1):
    """This toolchain's walrus accepts at most 1 sync wait and 1 update per
    instruction. Move excess waits onto same-engine prefix NoOps (the engine
    is in-order, so waiting earlier is equivalent) and excess updates onto
    suffix NoOps (signalling marginally later is safe)."""
    n = 0
    for func in nc.m.functions:
        for block in func.blocks:
            lst = block.instructions
            new = []
            for inst in lst:
                si = inst.sync_info
                ow = list(si.on_wait) if (si and si.on_wait) else []
                if len(ow) > max_w:
                    extra, keep = ow[:-max_w], ow[-max_w:]
                    for k in range(0, len(extra), max_w):
                        nop = mybir.InstNoOp(name=f"I-wsplit-{n}"); n += 1
                        nop.engine = inst.engine
                        nop.sync_info = mybir.SyncInfo(
                            on_wait=extra[k:k + max_w], on_update=[])
                        new.append(nop)
                    si.on_wait = keep
                new.append(inst)
                ou = list(si.on_update) if (si and si.on_update) else []
                if len(ou) > max_u:
                    keep_u, extra_u = ou[:max_u], ou[max_u:]
                    si.on_update = keep_u
                    for k in range(0, len(extra_u), max_u):
                        nop = mybir.InstNoOp(name=f"I-usplit-{n}"); n += 1
                        nop.engine = inst.engine
                        nop.sync_info = mybir.SyncInfo(
                            on_wait=[], on_update=extra_u[k:k + max_u])
                        new.append(nop)
            lst[:] = new
    return n


_NC_CACHE = None


def _get_nc():
    global _NC_CACHE
    if _NC_CACHE is None:
        _NC_CACHE = _build_nc()
    return _NC_CACHE


# ============================================================================
# Host combine
# ============================================================================

def _host_combine(core_outs):
    """core_outs: list of 8 dicts with device outputs. Returns scalar loss f32."""
    f32 = np.float32
    cd_sum = np.float64(0.0)
    rep_sum = np.float64(0.0)
    covs_p = []
    covs_g = []
    r2 = f32(REP_THRESH * REP_THRESH)
    for co in core_outs:
        rowmax = np.asarray(co["rowmax_pg"], dtype=f32)   # [128, 2*NB]: col 2b+h
        colmax = np.asarray(co["colmax_pg"], dtype=f32)   # [1, N]
        rowfull = rowmax.reshape(128, NB, 2).max(axis=2)  # max over column halves
        cd_sum += (-rowfull).sum(dtype=np.float64) + (-colmax).sum(dtype=np.float64)
        s1 = np.asarray(co["s1_out"], dtype=f32).T.reshape(-1)  # [NB*128]? careful below
        s2 = np.asarray(co["s2_out"], dtype=f32).T.reshape(-1)
        # s1/s2 layout [128 rows-in-block, NB blocks] -> row index = b*128 + p
        # transpose -> [NB, 128] -> flatten = global row order
        # per-row active recovery: 0, 1 or 2(+) actives
        # Recover the (<=2 per row) active relu terms from the two moments:
        # a+b = s1, a^2+b^2 = s2  ->  a,b = (s1 +- sqrt(2*s2 - s1^2))/2.
        # One-active rows fall out naturally (b ~ 0 -> zero contribution).
        with np.errstate(invalid="ignore"):
            disc = np.maximum(2 * s2 - s1 * s1, 0.0)
            sq = np.sqrt(disc)
            va = np.minimum((s1 + sq) * 0.5, r2)
            vb = np.maximum((s1 - sq) * 0.5, 0.0)
        act1 = s1 > 0
        da = np.sqrt(np.maximum(r2 - va, 1e-12))
        db = np.sqrt(np.maximum(r2 - vb, 1e-12))
        contrib = np.maximum(REP_THRESH - da, 0.0) + np.maximum(REP_THRESH - db, 0.0)
        rep_sum += contrib[act1].sum(dtype=np.float64)
        covs_p.append(np.asarray(co["cov_p"], dtype=f32))
        covs_g.append(np.asarray(co["cov_g"], dtype=f32))

    cd = cd_sum / (B * N)  # both directions summed /(B*N) each -> here N==M
    rep = rep_sum / (B * N * K_REP)

    def covs_to_normals(cov10_list):
        # cov10: [10, N] rows [x2,xy,xz,y2,yz,z2,x,y,z,1] (sums incl self)
        allc = np.concatenate([c[None] for c in cov10_list], 0)  # [B, 10, N]
        cnt = allc[:, 9, :]
        cnt = np.maximum(cnt, 1.0)
        mu = allc[:, 6:9, :] / cnt[:, None, :]         # [B, 3, N]
        M2 = allc[:, 0:6, :] / cnt[:, None, :]
        cov = np.empty((allc.shape[0], allc.shape[2], 3, 3), dtype=f32)
        xx_, xy_, xz_, yy_, yz_, zz_ = (M2[:, i, :] for i in range(6))
        mx, my, mz = mu[:, 0], mu[:, 1], mu[:, 2]
        cov[:, :, 0, 0] = xx_ - mx * mx
        cov[:, :, 0, 1] = cov[:, :, 1, 0] = xy_ - mx * my
        cov[:, :, 0, 2] = cov[:, :, 2, 0] = xz_ - mx * mz
        cov[:, :, 1, 1] = yy_ - my * my
        cov[:, :, 1, 2] = cov[:, :, 2, 1] = yz_ - my * mz
        cov[:, :, 2, 2] = zz_ - mz * mz
        return eigh3_smallest_lapack(cov.reshape(-1, 3, 3).astype(np.float32))

    n_p = covs_to_normals(covs_p)
    n_g = covs_to_normals(covs_g)
    dots = (n_p * n_g).sum(-1)
    normc = 1.0 - dots.mean(dtype=np.float64)

    loss = CD_W * cd + REP_W * rep + NORM_W * normc
    return np.float32(loss)


# ============================================================================
# Entry point
# ============================================================================

def kernel(pred, gt):
    pred = np.asarray(pred, dtype=np.float32)
    gt = np.asarray(gt, dtype=np.float32)
    assert pred.shape == (B, N, DIM) and gt.shape == (B, N, DIM)

    in_maps = [_prep_core_inputs(pred[c], gt[c]) for c in range(B)]

    from concourse.bass_utils import run_bass_kernel_spmd
    nc = _get_nc()
    res = run_bass_kernel_spmd(nc, in_maps, core_ids=list(range(8)))
    core_outs = res.results
    return _host_combine(core_outs)


if __name__ == "__main__":
    rng = np.random.default_rng(0)
    pred = rng.uniform(size=(B, N, DIM)).astype(np.float32)
    gt = rng.uniform(size=(B, N, DIM)).astype(np.float32)
    print("loss:", kernel(pred, gt))



# revision 7
# speedup vs baseline: 7.5382x; 7.5382x over previous
"""Trainium2 Bass kernel for nn_CombinedLoss (chamfer + repulsion + PCA-normal
consistency) on point clouds [8, 2048, 3].

Sharding: data-parallel over batch B=8 across 8 NeuronCores (1 sample/core).

v2 — restructured for the axon tunnel's ~80ms/RPC latency:
  - raw pred/gt uploaded (384KB total); ALL input prep happens on device
    (squared norms, fp32 distance-matmul operand rows, feature rows and
    their bf16 hi/lo transposed layout for the covariance matmul)
  - distance matrices -D via fp32 PE matmuls with K=5 augmented contraction
  - chamfer row/col reductions and the repulsion moment inversion are
    reduced to per-core SCALARS on device
  - everything is written to ONE fused output tensor [10, 4160] per core
    (cov moments of both clouds + 4 scalars) -> a single fetch round trip
  - the jitted shard_map executable is built once and cached; the donated
    output-zero buffer is recycled from the previous call's output
Host: centering + batched np.linalg.eigh (threaded) for the smallest-eigval
eigenvectors, then the weighted loss.
"""

import numpy as np
from concurrent.futures import ThreadPoolExecutor

try:
    import ml_dtypes

    BF16 = ml_dtypes.bfloat16
except Exception:  # pragma: no cover
    BF16 = None

B, N, DIM = 8, 2048, 3
K_REP = 4
REP_THRESH = np.float32(0.02)
R2 = float(np.float32(REP_THRESH) * np.float32(REP_THRESH))
K_NORM = 16
CD_W, REP_W, NORM_W = 1.0, 0.1, 0.01
NB = N // 128  # 16 row blocks
NEG_BIG = np.float32(-1e30)
SCAL_OFF = 2 * N          # scalar block starts here
OUT_COLS = 2 * N + 64     # fused output [10, OUT_COLS] f32 per core


# ============================================================================
# Bass device kernel builder
# ============================================================================

def _build_nc(split_waits=True):
    import concourse.bass as bass
    import concourse.mybir as mybir
    from concourse.tile import TileContext

    f32 = mybir.dt.float32
    bf16 = mybir.dt.bfloat16
    Alu = mybir.AluOpType
    Act = mybir.ActivationFunctionType
    Axis = mybir.AxisListType

    nc = bass.Bass()

    # ---- DRAM io (declaration order == jit operand order) ----
    pred = nc.dram_tensor("pred", [N, DIM], f32, kind="ExternalInput")
    gt = nc.dram_tensor("gt", [N, DIM], f32, kind="ExternalInput")
    ident = nc.dram_tensor("ident", [128, 128], bf16, kind="ExternalInput")
    negdiag = nc.dram_tensor("negdiag", [128, 128], bf16, kind="ExternalInput")
    out = nc.dram_tensor("out", [10, OUT_COLS], f32, kind="ExternalOutput")

    with TileContext(nc) as tc:
        import contextlib
        ctx = contextlib.ExitStack()
        with ctx:
            persist = ctx.enter_context(tc.tile_pool(name="persist", bufs=1))
            big = ctx.enter_context(tc.tile_pool(name="big", bufs=1))
            scrp = ctx.enter_context(tc.tile_pool(name="scr", bufs=1))
            ndmp = ctx.enter_context(tc.tile_pool(name="ndm", bufs=2))
            wtp = ctx.enter_context(tc.tile_pool(name="wtp", bufs=2))
            psd = ctx.enter_context(tc.tile_pool(name="psd", bufs=2, space="PSUM"))
            psc = ctx.enter_context(tc.tile_pool(name="psc", bufs=1, space="PSUM"))

            # ---- consts ----
            t_ident = persist.tile([128, 128], bf16, tag="ident")
            t_negdiag = persist.tile([128, 128], bf16, tag="ndg")
            nc.sync.dma_start(t_ident[:], ident[:])
            nc.sync.dma_start(t_negdiag[:], negdiag[:])
            t_ones = persist.tile([128, 128], bf16, tag="ones")
            nc.vector.memset(t_ones[:], 1.0)
            t_bias4 = persist.tile([128, 1], f32, tag="bias4")
            t_bias0 = persist.tile([128, 1], f32, tag="bias0")
            nc.vector.memset(t_bias4[:], R2)
            nc.vector.memset(t_bias0[:], 0.0)

            # ---- persistent per-cloud operand tiles ----
            A5 = {}; W5 = {}; F10 = {}; FT = {}
            for cl in ("p", "g"):
                A5[cl] = persist.tile([5, N], f32, tag=f"A5{cl}", name=f"A5{cl}")
                W5[cl] = persist.tile([5, N], f32, tag=f"W5{cl}", name=f"W5{cl}")
                F10[cl] = persist.tile([10, N], f32, tag=f"F10{cl}", name=f"F10{cl}")
                FT[cl] = persist.tile([128, NB * 20], bf16, tag=f"FT{cl}",
                                      name=f"FT{cl}")

            t_rowmax = persist.tile([128, NB, 2], f32, tag="rowmax")
            t_s1 = persist.tile([128, NB], f32, tag="s1")
            t_s2 = persist.tile([128, NB], f32, tag="s2")

            # ================= on-device prep =================
            # A5 = [2x, 2y, 2z, nn, 1] (fp32 matmul lhs rows)
            # W5 = [x, y, z, -1, -nn]  (fp32 matmul rhs rows)
            # F10 = [x2,xy,xz,y2,yz,z2,x,y,z,1] of centered coords
            # FT  = transposed bf16 hi/lo features [128, kb*20 + (0:10 hi|10:20 lo)]
            def prep(src_dram, cl):
                P3 = scrp.tile([3, N], f32, tag="P3")
                nc.sync.dma_start(P3[:], src_dram[:].rearrange("a b -> b a"))
                S3 = scrp.tile([3, N], f32, tag="S3")
                nc.vector.tensor_tensor(S3[:], P3[:], P3[:], Alu.mult)
                r1 = scrp.tile([1, N], f32, tag="r1")
                r2t = scrp.tile([1, N], f32, tag="r2t")
                nc.sync.dma_start(r1[:], S3[1:2, :])
                nc.sync.dma_start(r2t[:], S3[2:3, :])
                nn = scrp.tile([1, N], f32, tag="nn")
                nc.vector.tensor_tensor(nn[:], S3[0:1, :], r1[:], Alu.add)
                nc.vector.tensor_tensor(nn[:], nn[:], r2t[:], Alu.add)
                # engine ops may only start at partitions {0,32,64,96}: memset
                # the whole tile for the constant rows, DMA the odd-row writes
                a5, w5 = A5[cl], W5[cl]
                nc.vector.memset(a5[:], 1.0)
                nc.scalar.activation(a5[0:3, :], P3[:], Act.Copy, scale=2.0)
                nc.sync.dma_start(a5[3:4, :], nn[:])
                nc.vector.memset(w5[:], -1.0)
                nc.vector.tensor_copy(w5[0:3, :], P3[:])
                nnn = scrp.tile([1, N], f32, tag="nnn")
                nc.scalar.activation(nnn[:], nn[:], Act.Copy, scale=-1.0)
                nc.sync.dma_start(w5[4:5, :], nnn[:])
                # centered features
                C3 = scrp.tile([3, N], f32, tag="C3")
                nc.vector.tensor_scalar_add(C3[:], P3[:], -0.5)
                A6 = scrp.tile([6, N], f32, tag="A6")
                B6 = scrp.tile([6, N], f32, tag="B6")
                # A6 rows = [c0,c0,c0,c1,c1,c2]; B6 rows = [c0,c1,c2,c1,c2,c2]
                nc.vector.tensor_copy(A6[0:1, :], C3[0:1, :])
                nc.sync.dma_start(A6[1:2, :], C3[0:1, :])
                nc.sync.dma_start(A6[2:3, :], C3[0:1, :])
                nc.sync.dma_start(A6[3:4, :], C3[1:2, :])
                nc.sync.dma_start(A6[4:5, :], C3[1:2, :])
                nc.sync.dma_start(A6[5:6, :], C3[2:3, :])
                nc.vector.tensor_copy(B6[0:3, :], C3[:])
                nc.sync.dma_start(B6[3:5, :], C3[1:3, :])
                nc.sync.dma_start(B6[5:6, :], C3[2:3, :])
                f10 = F10[cl]
                nc.vector.memset(f10[:], 1.0)
                nc.vector.tensor_tensor(f10[0:6, :], A6[:], B6[:], Alu.mult)
                nc.sync.dma_start(f10[6:9, :], C3[:])
                # bf16 hi/lo split of features
                hi10 = scrp.tile([10, N], bf16, tag="hi10")
                hif = scrp.tile([10, N], f32, tag="hif")
                lo10f = scrp.tile([10, N], f32, tag="lo10f")
                lo10 = scrp.tile([10, N], bf16, tag="lo10")
                nc.scalar.activation(hi10[:], f10[:], Act.Copy)
                nc.scalar.activation(hif[:], hi10[:], Act.Copy)
                nc.vector.tensor_tensor(lo10f[:], f10[:], hif[:], Alu.subtract)
                nc.scalar.activation(lo10[:], lo10f[:], Act.Copy)
                # transpose [10, 128]-chunks -> FT[:, kb*20 + 0:10 / 10:20]
                ftt = FT[cl]
                for b in range(NB):
                    csl = slice(b * 128, (b + 1) * 128)
                    pst = psd.tile([128, 16], bf16, tag="dps")
                    nc.tensor.transpose(pst[:, 0:10], hi10[:, csl],
                                        t_ident[0:10, 0:10])
                    nc.scalar.activation(ftt[:, b * 20:b * 20 + 10], pst[:, 0:10],
                                         Act.Copy)
                    pst2 = psd.tile([128, 16], bf16, tag="dps")
                    nc.tensor.transpose(pst2[:, 0:10], lo10[:, csl],
                                        t_ident[0:10, 0:10])
                    nc.scalar.activation(ftt[:, b * 20 + 10:b * 20 + 20],
                                         pst2[:, 0:10], Act.Copy)

            prep(pred, "p")
            prep(gt, "g")

            # fp32 distance matmul: psum[128, 1024] = -D block (row block b,
            # column half h) between clouds (a5 lhs, w5 rhs)
            def build_half(a5, w5, b, h, ps):
                for j in range(2):
                    nc.tensor.matmul(
                        ps[:, j * 512:(j + 1) * 512],
                        a5[:, b * 128:(b + 1) * 128],
                        w5[:, h * 1024 + j * 512:h * 1024 + (j + 1) * 512],
                        start=True, stop=True,
                    )

            # ================= phase 1: chamfer on -Dpg =================
            t_colacc = big.tile([128, N], f32, tag="bigA")
            t_colred = big.tile([128, N], f32, tag="bigB")
            for b in range(NB):
                for h in range(2):
                    ps = psd.tile([128, 1024], f32, tag="dps")
                    build_half(A5["p"], W5["g"], b, h, ps)
                    nc.vector.tensor_reduce(t_rowmax[:, b, h:h + 1],
                                            ps[:], Axis.X, Alu.max)
                    cslice = slice(h * 1024, (h + 1) * 1024)
                    if b == 0:
                        nc.vector.tensor_copy(t_colacc[:, cslice], ps[:])
                    else:
                        nc.vector.tensor_tensor(t_colacc[:, cslice],
                                                t_colacc[:, cslice], ps[:], Alu.max)
            # partition-tree max 128 -> 1 (DMA crosses partitions, DVE cannot)
            for h in [64, 32, 16, 8, 4, 2, 1]:
                nc.sync.dma_start(t_colred[0:h, :], t_colacc[h:2 * h, :])
                nc.vector.tensor_tensor(t_colacc[0:h, :], t_colacc[0:h, :],
                                        t_colred[0:h, :], Alu.max)
            # chamfer scalars: sum of per-row maxes + sum of col maxes (of -D)
            t_cdcol = persist.tile([1, 1], f32, tag="cdcol")
            nc.vector.tensor_reduce(t_cdcol[:], t_colacc[0:1, :], Axis.X, Alu.add)
            t_rowfull = scrp.tile([128, NB], f32, tag="rowfull")
            nc.vector.tensor_reduce(t_rowfull[:], t_rowmax[:], Axis.X, Alu.max)
            t_cdrow = persist.tile([128, 1], f32, tag="cdrow")
            nc.vector.tensor_reduce(t_cdrow[:], t_rowfull[:], Axis.X, Alu.add)

            # ================= phases 2-4 for pp and gg =================
            def normals_phase(cl, cov_off, do_rep):
                a5, w5, ftt, f10 = A5[cl], W5[cl], FT[cl], F10[cl]

                def build_ndm(b):
                    ndm = ndmp.tile([128, N], bf16, tag="ndm", name=f"ndm{cl}{b}")
                    for h in range(2):
                        ps = psd.tile([128, 1024], f32, tag="dps")
                        build_half(a5, w5, b, h, ps)
                        nc.scalar.activation(ndm[:, h * 1024:(h + 1) * 1024],
                                             ps[:], Act.Copy)
                    nc.vector.tensor_tensor(
                        ndm[:, b * 128:(b + 1) * 128],
                        ndm[:, b * 128:(b + 1) * 128],
                        t_negdiag[:], Alu.add)
                    return ndm

                # pass 1: repulsion moments + 16-NN radius (tau) per row
                t_tau = scrp.tile([128, NB], f32, tag="tau")
                for b in range(NB):
                    ndm = build_ndm(b)
                    if do_rep:
                        scr = scrp.tile([128, N], bf16, tag="repscr")
                        scr2 = scrp.tile([128, N], bf16, tag="repscr2")
                        nc.scalar.activation(scr[:], ndm[:], Act.Relu,
                                             bias=t_bias4[:],
                                             accum_out=t_s1[:, b:b + 1])
                        nc.scalar.activation(scr2[:], scr[:], Act.Square,
                                             bias=t_bias0[:],
                                             accum_out=t_s2[:, b:b + 1])
                    t1 = scrp.tile([128, 1024], bf16, tag="tree1")
                    At = scrp.tile([128, 512], bf16, tag="treeA")
                    At2 = scrp.tile([128, 512], bf16, tag="treeA2")
                    m8a = scrp.tile([128, 8], bf16, tag="m8a")
                    m8b = scrp.tile([128, 8], bf16, tag="m8b")
                    nc.vector.tensor_tensor(t1[:], ndm[:, 0:1024],
                                            ndm[:, 1024:2048], Alu.max)
                    nc.vector.tensor_tensor(At[:], t1[:, 0:512],
                                            t1[:, 512:1024], Alu.max)
                    nc.vector.max(m8a[:], At[:])
                    nc.vector.match_replace(At2[:], m8a[:], At[:], float(NEG_BIG))
                    nc.vector.max(m8b[:], At2[:])
                    nc.vector.tensor_copy(t_tau[:, b:b + 1], m8b[:, 6:7])

                # tau broadcast: per-row tau -> [1, N] row -> PE ones-matmul
                # broadcast across partitions; mask compare is then direct on
                # the SYMMETRIC ndm blocks: wt[j, i] = (ndm[j, i] >= tau_i)
                t_taub = scrp.tile([128, 128], bf16, tag="taub")
                nc.vector.memset(t_taub[:], 0.0)
                nc.vector.tensor_copy(t_taub[:, 0:NB], t_tau[:])
                ps_tt = psd.tile([128, 128], bf16, tag="dps")
                nc.tensor.transpose(ps_tt[:], t_taub[:], t_ident[:])
                t_tt = scrp.tile([NB, 128], bf16, tag="tts")
                nc.scalar.activation(t_tt[:], ps_tt[0:NB, :], Act.Copy)
                t_tauT = scrp.tile([128, N], bf16, tag="tauT")
                nc.vector.memset(t_tauT[:], 0.0)
                nc.sync.dma_start(t_tauT[0:1, :], t_tt[:])
                t_taubc = scrp.tile([128, N], bf16, tag="taubc")
                for h in range(2):
                    ps_tau = psd.tile([128, 1024], f32, tag="dps")
                    for bb in range(8):
                        c0 = h * 1024 + bb * 128
                        nc.tensor.matmul(ps_tau[:, bb * 128:(bb + 1) * 128],
                                         t_ones[:],
                                         t_tauT[:, c0:c0 + 128],
                                         start=True, stop=True)
                    nc.scalar.activation(t_taubc[:, h * 1024:(h + 1) * 1024],
                                         ps_tau[:], Act.Copy)

                # pass 2: rebuild -D per block, mask, accumulate covariance
                # moments cps[10, N] over kb (hi+lo)
                cps = psc.tile([10, N], f32, tag="cps")
                for kb in range(NB):
                    ndm = build_ndm(kb)
                    wt = wtp.tile([128, N], bf16, tag="wt", name=f"wt{cl}{kb}")
                    nc.vector.tensor_tensor(wt[:], ndm[:], t_taubc[:], Alu.is_ge)
                    for j in range(4):
                        cols = slice(j * 512, (j + 1) * 512)
                        for half in range(2):
                            nc.tensor.matmul(
                                cps[:, cols],
                                ftt[:, kb * 20 + half * 10:kb * 20 + (half + 1) * 10],
                                wt[:, cols],
                                start=(kb == 0 and half == 0),
                                stop=(kb == NB - 1 and half == 1))
                # self add + DMA to fused out
                covsb = big.tile([10, N], f32, tag="bigA", name=f"covsb{cl}")
                nc.vector.tensor_tensor(covsb[:], cps[:], f10[:], Alu.add)
                nc.sync.dma_start(out[:, cov_off:cov_off + N], covsb[:])

            normals_phase("p", 0, do_rep=True)

            # ---- repulsion moment inversion -> per-row contribution ----
            # a,b = (s1 +- sqrt(2*s2 - s1^2))/2; d=sqrt(r2-v); contrib =
            # relu(0.02-da)+relu(0.02-db), gated by s1>0
            sh = [128, NB]
            t_t1 = scrp.tile(sh, f32, tag="rp1")
            t_t2 = scrp.tile(sh, f32, tag="rp2")
            t_sq = scrp.tile(sh, f32, tag="rp3")
            t_va = scrp.tile(sh, f32, tag="rp4")
            t_vb = scrp.tile(sh, f32, tag="rp5")
            t_ca = scrp.tile(sh, f32, tag="rp6")
            t_cb = scrp.tile(sh, f32, tag="rp7")
            t_msk = scrp.tile(sh, f32, tag="rp8")
            Alu_ = Alu
            nc.vector.tensor_tensor(t_t1[:], t_s1[:], t_s1[:], Alu_.mult)
            nc.vector.tensor_scalar(t_t2[:], t_s2[:], 2.0, None, Alu_.mult)
            nc.vector.tensor_tensor(t_t2[:], t_t2[:], t_t1[:], Alu_.subtract)
            nc.vector.tensor_scalar_max(t_t2[:], t_t2[:], 0.0)
            nc.scalar.activation(t_sq[:], t_t2[:], Act.Sqrt)
            nc.vector.tensor_tensor(t_va[:], t_s1[:], t_sq[:], Alu_.add)
            nc.vector.tensor_scalar(t_va[:], t_va[:], 0.5, R2, Alu_.mult, Alu_.min)
            nc.vector.tensor_tensor(t_vb[:], t_s1[:], t_sq[:], Alu_.subtract)
            nc.vector.tensor_scalar(t_vb[:], t_vb[:], 0.5, 0.0, Alu_.mult, Alu_.max)
            # da = sqrt(max(r2 - va, 1e-12)); contrib_a = max(0.02 - da, 0)
            for tv, tc_ in ((t_va, t_ca), (t_vb, t_cb)):
                nc.vector.tensor_scalar(tv[:], tv[:], -1.0, R2, Alu_.mult, Alu_.add)
                nc.vector.tensor_scalar_max(tv[:], tv[:], 1e-12)
                nc.scalar.activation(tv[:], tv[:], Act.Sqrt)
                nc.vector.tensor_scalar(tc_[:], tv[:], -1.0, float(REP_THRESH),
                                        Alu_.mult, Alu_.add)
                nc.vector.tensor_scalar_max(tc_[:], tc_[:], 0.0)
            nc.vector.tensor_scalar(t_msk[:], t_s1[:], 0.0, None, Alu_.is_gt)
            nc.vector.tensor_tensor(t_ca[:], t_ca[:], t_cb[:], Alu_.add)
            nc.vector.tensor_tensor(t_ca[:], t_ca[:], t_msk[:], Alu_.mult)
            t_reprow = persist.tile([128, 1], f32, tag="reprow")
            nc.vector.tensor_reduce(t_reprow[:], t_ca[:], Axis.X, Alu_.add)

            # ---- partition-sum [cd_row, rep] via DMA tree; pack scalars ----
            t_P2 = scrp.tile([128, 2], f32, tag="P2")
            t_P2s = scrp.tile([64, 2], f32, tag="P2s")
            nc.vector.tensor_copy(t_P2[:, 0:1], t_cdrow[:])
            nc.vector.tensor_copy(t_P2[:, 1:2], t_reprow[:])
            for h in [64, 32, 16, 8, 4, 2, 1]:
                nc.sync.dma_start(t_P2s[0:h, :], t_P2[h:2 * h, :])
                nc.vector.tensor_tensor(t_P2[0:h, :], t_P2[0:h, :],
                                        t_P2s[0:h, :], Alu.add)
            t_zero10 = scrp.tile([10, 64], f32, tag="zero10")
            nc.vector.memset(t_zero10[:], 0.0)
            nc.vector.tensor_copy(t_zero10[0:1, 0:2], t_P2[0:1, :])
            nc.vector.tensor_copy(t_zero10[0:1, 2:3], t_cdcol[:])
            nc.sync.dma_start(out[:, SCAL_OFF:SCAL_OFF + 64], t_zero10[:])

            normals_phase("g", N, do_rep=False)

    if split_waits:
        _split_excess_waits(nc, mybir)
    return nc


def _split_excess_waits(nc, mybir, max_w=1, max_u=1):
    """This toolchain's walrus accepts at most 1 sync wait and 1 update per
    instruction. Move excess waits onto same-engine prefix NoOps (the engine
    is in-order, so waiting earlier is equivalent) and excess updates onto
    suffix NoOps (signalling marginally later is safe)."""
    n = 0
    for func in nc.m.functions:
        for block in func.blocks:
            lst = block.instructions
            new = []
            for inst in lst:
                si = inst.sync_info
                ow = list(si.on_wait) if (si and si.on_wait) else []
                if len(ow) > max_w:
                    extra, keep = ow[:-max_w], ow[-max_w:]
                    for k in range(0, len(extra), max_w):
                        nop = mybir.InstNoOp(name=f"I-wsplit-{n}"); n += 1
                        nop.engine = inst.engine
                        nop.sync_info = mybir.SyncInfo(
                            on_wait=extra[k:k + max_w], on_update=[])
                        new.append(nop)
                    si.on_wait = keep
                new.append(inst)
                ou = list(si.on_update) if (si and si.on_update) else []
                if len(ou) > max_u:
                    keep_u, extra_u = ou[:max_u], ou[max_u:]
                    si.on_update = keep_u
                    for k in range(0, len(extra_u), max_u):
                        nop = mybir.InstNoOp(name=f"I-usplit-{n}"); n += 1
                        nop.engine = inst.engine
                        nop.sync_info = mybir.SyncInfo(
                            on_wait=[], on_update=extra_u[k:k + max_u])
                        new.append(nop)
            lst[:] = new
    return n


_NC_CACHE = None


def _get_nc():
    global _NC_CACHE
    if _NC_CACHE is None:
        _NC_CACHE = _build_nc()
    return _NC_CACHE


def _consts_np():
    negdiag = np.zeros((128, 128), dtype=BF16)
    np.fill_diagonal(negdiag, BF16(NEG_BIG))
    ident = np.zeros((128, 128), dtype=BF16)
    np.fill_diagonal(ident, BF16(1.0))
    return ident, negdiag


# ============================================================================
# Cached jit runner (replicates bass2jax.run_bass_via_pjrt, but the jitted
# executable, mesh, and const device buffers are built ONCE; the donated
# output buffer is recycled from the previous call's output)
# ============================================================================

class _Runner:
    def __init__(self):
        import jax
        from jax.sharding import Mesh, PartitionSpec, NamedSharding
        from jax.experimental.shard_map import shard_map
        from concourse import bass2jax
        import concourse.mybir as mybir

        self.jax = jax
        nc = _get_nc()
        bass2jax.install_neuronx_cc_hook()

        partition_name = (nc.partition_id_tensor.name
                          if nc.partition_id_tensor else None)
        in_names, out_names, out_avals, zero_outs = [], [], [], []
        for alloc in nc.m.functions[0].allocations:
            if not isinstance(alloc, mybir.MemoryLocationSet):
                continue
            name = alloc.memorylocations[0].name
            if alloc.kind == "ExternalInput":
                if name != partition_name:
                    in_names.append(name)
            elif alloc.kind == "ExternalOutput":
                shape = tuple(alloc.tensor_shape)
                dtype = mybir.dt.np(alloc.dtype)
                out_names.append(name)
                out_avals.append(jax.core.ShapedArray(shape, dtype))
                zero_outs.append((shape, dtype))
        assert in_names == ["pred", "gt", "ident", "negdiag"], in_names
        assert out_names == ["out"], out_names
        n_params = len(in_names)
        n_outs = len(out_names)
        all_names = in_names + out_names
        if partition_name is not None:
            all_names.append(partition_name)
        self.out_shape = zero_outs[0][0]

        def _body(*args):
            operands = list(args)
            if partition_name is not None:
                operands.append(bass2jax.partition_id_tensor())
            outs = bass2jax._bass_exec_p.bind(
                *operands,
                out_avals=tuple(out_avals),
                in_names=tuple(all_names),
                out_names=tuple(out_names),
                lowering_input_output_aliases=(),
                sim_require_finite=True,
                sim_require_nnan=True,
                nc=nc,
            )
            return tuple(outs)

        devices = jax.devices()[:B]
        assert len(devices) == B, f"need {B} devices, have {len(jax.devices())}"
        mesh = Mesh(np.asarray(devices), ("core",))
        pspec = PartitionSpec("core")
        self._fn = jax.jit(
            shard_map(_body, mesh=mesh,
                      in_specs=(pspec,) * (n_params + n_outs),
                      out_specs=(pspec,) * n_outs,
                      check_rep=False),
            donate_argnums=(n_params,),
            keep_unused=True,
        )
        ident, negdiag = _consts_np()
        sh = NamedSharding(mesh, pspec)
        self._ident = jax.device_put(np.tile(ident, (B, 1)), sh)
        self._negdiag = jax.device_put(np.tile(negdiag, (B, 1)), sh)
        self._donate = None  # recycled output buffer

    def run(self, pred, gt):
        """pred, gt: [B, N, 3] f32 -> host np.ndarray [B*10, OUT_COLS] f32."""
        zeros = self._donate
        if zeros is None:
            zeros = np.zeros((B * self.out_shape[0],) + self.out_shape[1:],
                             np.float32)
        out, = self._fn(pred.reshape(B * N, DIM), gt.reshape(B * N, DIM),
                        self._ident, self._negdiag, zeros)
        arr = np.asarray(out)
        # the kernel writes every element of `out`, so last call's output
        # can be donated as the next call's output buffer
        self._donate = out
        return arr


_RUNNER = None


def _get_runner():
    global _RUNNER
    if _RUNNER is None:
        _RUNNER = _Runner()
    return _RUNNER


# ============================================================================
# Host combine
# ============================================================================

_EIGH_POOL = ThreadPoolExecutor(max_workers=8)


def _normals_from_moments(mo):
    """mo: [B, 10, N] f32 moment rows [x2,xy,xz,y2,yz,z2,x,y,z,cnt] (sums
    incl self) -> [B*N, 3] smallest-eigval eigenvectors via np.linalg.eigh."""
    f32 = np.float32
    cnt = np.maximum(mo[:, 9, :], 1.0)[:, None, :]
    mu = mo[:, 6:9, :] / cnt
    M2 = mo[:, 0:6, :] / cnt
    cov = np.empty((mo.shape[0], mo.shape[2], 3, 3), dtype=f32)
    xx_, xy_, xz_, yy_, yz_, zz_ = (M2[:, i, :] for i in range(6))
    mx, my, mz = mu[:, 0], mu[:, 1], mu[:, 2]
    cov[:, :, 0, 0] = xx_ - mx * mx
    cov[:, :, 0, 1] = cov[:, :, 1, 0] = xy_ - mx * my
    cov[:, :, 0, 2] = cov[:, :, 2, 0] = xz_ - mx * mz
    cov[:, :, 1, 1] = yy_ - my * my
    cov[:, :, 1, 2] = cov[:, :, 2, 1] = yz_ - my * mz
    cov[:, :, 2, 2] = zz_ - mz * mz
    cov = cov.reshape(-1, 3, 3)
    nchunk = 8
    chunks = np.array_split(cov, nchunk)
    outs = list(_EIGH_POOL.map(lambda c: np.linalg.eigh(c)[1][:, :, 0], chunks))
    return np.concatenate(outs, 0)


def _host_combine(arr):
    """arr: [B*10, OUT_COLS] f32 device output -> scalar loss f32."""
    a = arr.reshape(B, 10, OUT_COLS)
    scal = a[:, 0, SCAL_OFF:SCAL_OFF + 4].astype(np.float64)
    cd = -(scal[:, 0].sum() + scal[:, 2].sum()) / (B * N)
    rep = scal[:, 1].sum() / (B * N * K_REP)

    fp = _EIGH_POOL.submit(_normals_from_moments, a[:, :, 0:N])
    n_g = _normals_from_moments(a[:, :, N:2 * N])
    n_p = fp.result()
    dots = (n_p * n_g).sum(-1)
    normc = 1.0 - dots.mean(dtype=np.float64)

    return np.float32(CD_W * cd + REP_W * rep + NORM_W * normc)


# ============================================================================
# Entry point
# ============================================================================

def kernel(pred, gt):
    pred = np.ascontiguousarray(np.asarray(pred, dtype=np.float32))
    gt = np.ascontiguousarray(np.asarray(gt, dtype=np.float32))
    assert pred.shape == (B, N, DIM) and gt.shape == (B, N, DIM)
    arr = _get_runner().run(pred, gt)
    return _host_combine(arr)


if __name__ == "__main__":
    rng = np.random.default_rng(0)
    pred = rng.uniform(size=(B, N, DIM)).astype(np.float32)
    gt = rng.uniform(size=(B, N, DIM)).astype(np.float32)
    print("loss:", kernel(pred, gt))


# revision 17
# speedup vs baseline: 8.9961x; 1.1934x over previous
"""Trainium2 Bass kernel for nn_CombinedLoss (chamfer + repulsion + PCA-normal
consistency) on point clouds [8, 2048, 3].

Sharding: data-parallel over batch B=8 across 8 NeuronCores (1 sample/core).

v2 — restructured for the axon tunnel's ~80ms/RPC latency:
  - raw pred/gt uploaded (384KB total); ALL input prep happens on device
    (squared norms, fp32 distance-matmul operand rows, feature rows and
    their bf16 hi/lo transposed layout for the covariance matmul)
  - distance matrices -D via fp32 PE matmuls with K=5 augmented contraction
  - chamfer row/col reductions and the repulsion moment inversion are
    reduced to per-core SCALARS on device
  - everything is written to ONE fused output tensor [10, 4160] per core
    (cov moments of both clouds + 4 scalars) -> a single fetch round trip
  - the jitted shard_map executable is built once and cached; the donated
    output-zero buffer is recycled from the previous call's output
Host: centering + batched np.linalg.eigh (threaded) for the smallest-eigval
eigenvectors, then the weighted loss.
"""

import numpy as np
from concurrent.futures import ThreadPoolExecutor

try:
    import ml_dtypes

    BF16 = ml_dtypes.bfloat16
except Exception:  # pragma: no cover
    BF16 = None

B, N, DIM = 8, 2048, 3
K_REP = 4
REP_THRESH = np.float32(0.02)
R2 = float(np.float32(REP_THRESH) * np.float32(REP_THRESH))
K_NORM = 16
CD_W, REP_W, NORM_W = 1.0, 0.1, 0.01
NB = N // 128  # 16 row blocks
NEG_BIG = np.float32(-1e30)
SCAL_OFF = 2 * N          # scalar block starts here
OUT_COLS = 2 * N + 64     # fused output [10, OUT_COLS] f32 per core


# ============================================================================
# Bass device kernel builder
# ============================================================================

def _build_nc(split_waits=True):
    import concourse.bass as bass
    import concourse.mybir as mybir
    from concourse.tile import TileContext

    f32 = mybir.dt.float32
    bf16 = mybir.dt.bfloat16
    Alu = mybir.AluOpType
    Act = mybir.ActivationFunctionType
    Axis = mybir.AxisListType

    nc = bass.Bass()

    # ---- DRAM io (declaration order == jit operand order) ----
    f16 = mybir.dt.float16
    pred = nc.dram_tensor("pred", [N, DIM], f32, kind="ExternalInput")
    gt = nc.dram_tensor("gt", [N, DIM], f32, kind="ExternalInput")
    ident = nc.dram_tensor("ident", [128, 128], bf16, kind="ExternalInput")
    negdiag = nc.dram_tensor("negdiag", [128, 128], bf16, kind="ExternalInput")
    # centered covariance entries [xx,xy,xz,yy,yz,zz] per point, f16;
    # out_g additionally carries the 4 scalars in its last 32 columns
    out_p = nc.dram_tensor("out_p", [6, N], f16, kind="ExternalOutput")
    out_g = nc.dram_tensor("out_g", [6, N + 32], f16, kind="ExternalOutput")

    with TileContext(nc) as tc:
        import contextlib
        ctx = contextlib.ExitStack()
        with ctx:
            persist = ctx.enter_context(tc.tile_pool(name="persist", bufs=1))
            big = ctx.enter_context(tc.tile_pool(name="big", bufs=1))
            scrp = ctx.enter_context(tc.tile_pool(name="scr", bufs=1))
            ndmp = ctx.enter_context(tc.tile_pool(name="ndm", bufs=2))
            wtp = ctx.enter_context(tc.tile_pool(name="wtp", bufs=2))
            psd = ctx.enter_context(tc.tile_pool(name="psd", bufs=2, space="PSUM"))
            psc = ctx.enter_context(tc.tile_pool(name="psc", bufs=1, space="PSUM"))

            # ---- consts ----
            t_ident = persist.tile([128, 128], bf16, tag="ident")
            t_negdiag = persist.tile([128, 128], bf16, tag="ndg")
            nc.sync.dma_start(t_ident[:], ident[:])
            nc.sync.dma_start(t_negdiag[:], negdiag[:])
            t_ones = persist.tile([128, 128], bf16, tag="ones")
            nc.vector.memset(t_ones[:], 1.0)
            t_bias4 = persist.tile([128, 1], f32, tag="bias4")
            t_bias0 = persist.tile([128, 1], f32, tag="bias0")
            nc.vector.memset(t_bias4[:], R2)
            nc.vector.memset(t_bias0[:], 0.0)
            t_ones6 = persist.tile([1, 8], f32, tag="ones6")
            nc.vector.memset(t_ones6[:], 1.0)

            # ---- persistent per-cloud operand tiles ----
            A5 = {}; W5 = {}; F10 = {}; FT = {}
            for cl in ("p", "g"):
                A5[cl] = persist.tile([5, N], f32, tag=f"A5{cl}", name=f"A5{cl}")
                W5[cl] = persist.tile([5, N], f32, tag=f"W5{cl}", name=f"W5{cl}")
                F10[cl] = persist.tile([10, N], f32, tag=f"F10{cl}", name=f"F10{cl}")
                FT[cl] = persist.tile([128, NB * 20], bf16, tag=f"FT{cl}",
                                      name=f"FT{cl}")

            t_rowmax = persist.tile([128, NB, 2], f32, tag="rowmax")
            t_s1 = persist.tile([128, NB], f32, tag="s1")
            t_s2 = persist.tile([128, NB], f32, tag="s2")

            # ================= on-device prep =================
            # A5 = [2x, 2y, 2z, nn, 1] (fp32 matmul lhs rows)
            # W5 = [x, y, z, -1, -nn]  (fp32 matmul rhs rows)
            # F10 = [x2,xy,xz,y2,yz,z2,x,y,z,1] of centered coords
            # FT  = transposed bf16 hi/lo features [128, kb*20 + (0:10 hi|10:20 lo)]
            def prep(src_dram, cl):
                P3 = scrp.tile([3, N], f32, tag="P3")
                nc.sync.dma_start(P3[:], src_dram[:].rearrange("a b -> b a"))
                S3 = scrp.tile([3, N], f32, tag="S3")
                nc.vector.tensor_tensor(S3[:], P3[:], P3[:], Alu.mult)
                r1 = scrp.tile([1, N], f32, tag="r1")
                r2t = scrp.tile([1, N], f32, tag="r2t")
                nc.sync.dma_start(r1[:], S3[1:2, :])
                nc.sync.dma_start(r2t[:], S3[2:3, :])
                nn = scrp.tile([1, N], f32, tag="nn")
                nc.vector.tensor_tensor(nn[:], S3[0:1, :], r1[:], Alu.add)
                nc.vector.tensor_tensor(nn[:], nn[:], r2t[:], Alu.add)
                # engine ops may only start at partitions {0,32,64,96}: memset
                # the whole tile for the constant rows, DMA the odd-row writes
                a5, w5 = A5[cl], W5[cl]
                nc.vector.memset(a5[:], 1.0)
                nc.scalar.activation(a5[0:3, :], P3[:], Act.Copy, scale=2.0)
                nc.sync.dma_start(a5[3:4, :], nn[:])
                nc.vector.memset(w5[:], -1.0)
                nc.vector.tensor_copy(w5[0:3, :], P3[:])
                nnn = scrp.tile([1, N], f32, tag="nnn")
                nc.scalar.activation(nnn[:], nn[:], Act.Copy, scale=-1.0)
                nc.sync.dma_start(w5[4:5, :], nnn[:])
                # centered features
                C3 = scrp.tile([3, N], f32, tag="C3")
                nc.vector.tensor_scalar_add(C3[:], P3[:], -0.5)
                A6 = scrp.tile([6, N], f32, tag="A6")
                B6 = scrp.tile([6, N], f32, tag="B6")
                # A6 rows = [c0,c0,c0,c1,c1,c2]; B6 rows = [c0,c1,c2,c1,c2,c2]
                nc.vector.tensor_copy(A6[0:1, :], C3[0:1, :])
                nc.sync.dma_start(A6[1:2, :], C3[0:1, :])
                nc.sync.dma_start(A6[2:3, :], C3[0:1, :])
                nc.sync.dma_start(A6[3:4, :], C3[1:2, :])
                nc.sync.dma_start(A6[4:5, :], C3[1:2, :])
                nc.sync.dma_start(A6[5:6, :], C3[2:3, :])
                nc.vector.tensor_copy(B6[0:3, :], C3[:])
                nc.sync.dma_start(B6[3:5, :], C3[1:3, :])
                nc.sync.dma_start(B6[5:6, :], C3[2:3, :])
                f10 = F10[cl]
                nc.vector.memset(f10[:], 1.0)
                nc.vector.tensor_tensor(f10[0:6, :], A6[:], B6[:], Alu.mult)
                nc.sync.dma_start(f10[6:9, :], C3[:])
                # bf16 hi/lo split of features
                hi10 = scrp.tile([10, N], bf16, tag="hi10")
                hif = scrp.tile([10, N], f32, tag="hif")
                lo10f = scrp.tile([10, N], f32, tag="lo10f")
                lo10 = scrp.tile([10, N], bf16, tag="lo10")
                nc.scalar.activation(hi10[:], f10[:], Act.Copy)
                nc.scalar.activation(hif[:], hi10[:], Act.Copy)
                nc.vector.tensor_tensor(lo10f[:], f10[:], hif[:], Alu.subtract)
                nc.scalar.activation(lo10[:], lo10f[:], Act.Copy)
                # transpose [10, 128]-chunks -> FT[:, kb*20 + 0:10 / 10:20]
                ftt = FT[cl]
                for b in range(NB):
                    csl = slice(b * 128, (b + 1) * 128)
                    pst = psd.tile([128, 16], bf16, tag="dps")
                    nc.tensor.transpose(pst[:, 0:10], hi10[:, csl],
                                        t_ident[0:10, 0:10])
                    nc.scalar.activation(ftt[:, b * 20:b * 20 + 10], pst[:, 0:10],
                                         Act.Copy)
                    pst2 = psd.tile([128, 16], bf16, tag="dps")
                    nc.tensor.transpose(pst2[:, 0:10], lo10[:, csl],
                                        t_ident[0:10, 0:10])
                    nc.scalar.activation(ftt[:, b * 20 + 10:b * 20 + 20],
                                         pst2[:, 0:10], Act.Copy)

            prep(pred, "p")
            prep(gt, "g")

            # fp32 distance matmul: psum[128, 1024] = -D block (row block b,
            # column half h) between clouds (a5 lhs, w5 rhs)
            def build_half(a5, w5, b, h, ps):
                for j in range(2):
                    nc.tensor.matmul(
                        ps[:, j * 512:(j + 1) * 512],
                        a5[:, b * 128:(b + 1) * 128],
                        w5[:, h * 1024 + j * 512:h * 1024 + (j + 1) * 512],
                        start=True, stop=True,
                    )

            # ================= phase 1: chamfer on -Dpg =================
            t_colacc = big.tile([128, N], f32, tag="bigA")
            t_colred = big.tile([128, N], f32, tag="bigB")
            for b in range(NB):
                for h in range(2):
                    ps = psd.tile([128, 1024], f32, tag="dps")
                    build_half(A5["p"], W5["g"], b, h, ps)
                    nc.vector.tensor_reduce(t_rowmax[:, b, h:h + 1],
                                            ps[:], Axis.X, Alu.max)
                    cslice = slice(h * 1024, (h + 1) * 1024)
                    if b == 0:
                        nc.vector.tensor_copy(t_colacc[:, cslice], ps[:])
                    else:
                        nc.vector.tensor_tensor(t_colacc[:, cslice],
                                                t_colacc[:, cslice], ps[:], Alu.max)
            # partition-tree max 128 -> 1 (DMA crosses partitions, DVE cannot)
            for h in [64, 32, 16, 8, 4, 2, 1]:
                nc.sync.dma_start(t_colred[0:h, :], t_colacc[h:2 * h, :])
                nc.vector.tensor_tensor(t_colacc[0:h, :], t_colacc[0:h, :],
                                        t_colred[0:h, :], Alu.max)
            # chamfer scalars: sum of per-row maxes + sum of col maxes (of -D)
            t_cdcol = persist.tile([1, 1], f32, tag="cdcol")
            nc.vector.tensor_reduce(t_cdcol[:], t_colacc[0:1, :], Axis.X, Alu.add)
            t_rowfull = scrp.tile([128, NB], f32, tag="rowfull")
            nc.vector.tensor_reduce(t_rowfull[:], t_rowmax[:], Axis.X, Alu.max)
            t_cdrow = persist.tile([128, 1], f32, tag="cdrow")
            nc.vector.tensor_reduce(t_cdrow[:], t_rowfull[:], Axis.X, Alu.add)

            # ================= phases 2-4 for pp and gg =================
            def normals_phase(cl, out_dram, do_rep):
                a5, w5, ftt, f10 = A5[cl], W5[cl], FT[cl], F10[cl]

                def build_ndm(b):
                    ndm = ndmp.tile([128, N], bf16, tag="ndm", name=f"ndm{cl}{b}")
                    for h in range(2):
                        ps = psd.tile([128, 1024], f32, tag="dps")
                        build_half(a5, w5, b, h, ps)
                        nc.scalar.activation(ndm[:, h * 1024:(h + 1) * 1024],
                                             ps[:], Act.Copy)
                    nc.vector.tensor_tensor(
                        ndm[:, b * 128:(b + 1) * 128],
                        ndm[:, b * 128:(b + 1) * 128],
                        t_negdiag[:], Alu.add)
                    return ndm

                # pass 1: repulsion moments + 16-NN radius (tau) per row
                t_tau = scrp.tile([128, NB], f32, tag="tau")
                for b in range(NB):
                    ndm = build_ndm(b)
                    if do_rep:
                        scr = scrp.tile([128, N], bf16, tag="repscr")
                        scr2 = scrp.tile([128, N], bf16, tag="repscr2")
                        nc.scalar.activation(scr[:], ndm[:], Act.Relu,
                                             bias=t_bias4[:],
                                             accum_out=t_s1[:, b:b + 1])
                        nc.scalar.activation(scr2[:], scr[:], Act.Square,
                                             bias=t_bias0[:],
                                             accum_out=t_s2[:, b:b + 1])
                    t1 = scrp.tile([128, 1024], bf16, tag="tree1")
                    At = scrp.tile([128, 512], bf16, tag="treeA")
                    At2 = scrp.tile([128, 512], bf16, tag="treeA2")
                    m8a = scrp.tile([128, 8], bf16, tag="m8a")
                    m8b = scrp.tile([128, 8], bf16, tag="m8b")
                    nc.vector.tensor_tensor(t1[:], ndm[:, 0:1024],
                                            ndm[:, 1024:2048], Alu.max)
                    nc.vector.tensor_tensor(At[:], t1[:, 0:512],
                                            t1[:, 512:1024], Alu.max)
                    nc.vector.max(m8a[:], At[:])
                    nc.vector.match_replace(At2[:], m8a[:], At[:], float(NEG_BIG))
                    nc.vector.max(m8b[:], At2[:])
                    nc.vector.tensor_copy(t_tau[:, b:b + 1], m8b[:, 6:7])

                # tau broadcast: per-row tau -> [1, N] row -> PE ones-matmul
                # broadcast across partitions; mask compare is then direct on
                # the SYMMETRIC ndm blocks: wt[j, i] = (ndm[j, i] >= tau_i)
                t_taub = scrp.tile([128, 128], bf16, tag="taub")
                nc.vector.memset(t_taub[:], 0.0)
                nc.vector.tensor_copy(t_taub[:, 0:NB], t_tau[:])
                ps_tt = psd.tile([128, 128], bf16, tag="dps")
                nc.tensor.transpose(ps_tt[:], t_taub[:], t_ident[:])
                t_tt = scrp.tile([NB, 128], bf16, tag="tts")
                nc.scalar.activation(t_tt[:], ps_tt[0:NB, :], Act.Copy)
                t_tauT = scrp.tile([128, N], bf16, tag="tauT")
                nc.vector.memset(t_tauT[:], 0.0)
                nc.sync.dma_start(t_tauT[0:1, :], t_tt[:])
                t_taubc = scrp.tile([128, N], bf16, tag="taubc")
                for h in range(2):
                    ps_tau = psd.tile([128, 1024], f32, tag="dps")
                    for bb in range(8):
                        c0 = h * 1024 + bb * 128
                        nc.tensor.matmul(ps_tau[:, bb * 128:(bb + 1) * 128],
                                         t_ones[:],
                                         t_tauT[:, c0:c0 + 128],
                                         start=True, stop=True)
                    nc.scalar.activation(t_taubc[:, h * 1024:(h + 1) * 1024],
                                         ps_tau[:], Act.Copy)

                # pass 2: rebuild -D per block, mask, accumulate covariance
                # moments cps[10, N] over kb (hi+lo)
                cps = psc.tile([10, N], f32, tag="cps")
                for kb in range(NB):
                    ndm = build_ndm(kb)
                    wt = wtp.tile([128, N], bf16, tag="wt", name=f"wt{cl}{kb}")
                    nc.vector.tensor_tensor(wt[:], ndm[:], t_taubc[:], Alu.is_ge)
                    for j in range(4):
                        cols = slice(j * 512, (j + 1) * 512)
                        for half in range(2):
                            nc.tensor.matmul(
                                cps[:, cols],
                                ftt[:, kb * 20 + half * 10:kb * 20 + (half + 1) * 10],
                                wt[:, cols],
                                start=(kb == 0 and half == 0),
                                stop=(kb == NB - 1 and half == 1))
                # self add, then center on device:
                #   covc[ab] = M2[ab]/cnt - (s[a]/cnt)*(s[b]/cnt)   (f16 out)
                covsb = big.tile([10, N], f32, tag="bigA", name=f"covsb{cl}")
                nc.vector.tensor_tensor(covsb[:], cps[:], f10[:], Alu.add)
                rr = scrp.tile([1, N], f32, tag="r1")
                nc.sync.dma_start(rr[:], covsb[9:10, :])
                rcp = scrp.tile([1, N], f32, tag="r2t")
                nc.vector.reciprocal(rcp[:], rr[:])
                mus = scrp.tile([3, N], f32, tag="S3")
                nc.sync.dma_start(mus[:], covsb[6:9, :])
                psB3 = psc.tile([3, N], f32, tag="cps", name=f"psB3{cl}")
                for j in range(4):
                    cj = slice(j * 512, (j + 1) * 512)
                    nc.tensor.matmul(psB3[:, cj], t_ones6[0:1, 0:3], rcp[:, cj],
                                     start=True, stop=True)
                mu3 = scrp.tile([3, N], f32, tag="C3")
                nc.vector.tensor_tensor(mu3[:], mus[:], psB3[:], Alu.mult)
                A6m = scrp.tile([6, N], f32, tag="A6")
                B6m = scrp.tile([6, N], f32, tag="B6")
                nc.vector.tensor_copy(A6m[0:1, :], mu3[0:1, :])
                nc.sync.dma_start(A6m[1:2, :], mu3[0:1, :])
                nc.sync.dma_start(A6m[2:3, :], mu3[0:1, :])
                nc.sync.dma_start(A6m[3:4, :], mu3[1:2, :])
                nc.sync.dma_start(A6m[4:5, :], mu3[1:2, :])
                nc.sync.dma_start(A6m[5:6, :], mu3[2:3, :])
                nc.vector.tensor_copy(B6m[0:3, :], mu3[:])
                nc.sync.dma_start(B6m[3:5, :], mu3[1:3, :])
                nc.sync.dma_start(B6m[5:6, :], mu3[2:3, :])
                P6 = scrp.tile([6, N], f32, tag="lo10f")
                nc.vector.tensor_tensor(P6[:], A6m[:], B6m[:], Alu.mult)
                psB6 = psc.tile([6, N], f32, tag="cps", name=f"psB6{cl}")
                for j in range(4):
                    cj = slice(j * 512, (j + 1) * 512)
                    nc.tensor.matmul(psB6[:, cj], t_ones6[0:1, 0:6], rcp[:, cj],
                                     start=True, stop=True)
                M2r = scrp.tile([6, N], f32, tag="hif")
                nc.vector.tensor_tensor(M2r[:], covsb[0:6, :], psB6[:], Alu.mult)
                covc = scrp.tile([6, N], f16, tag="hi10")
                nc.vector.tensor_tensor(covc[:], M2r[:], P6[:], Alu.subtract)
                nc.sync.dma_start(out_dram[:, 0:N], covc[:])

            normals_phase("p", out_p, do_rep=True)

            # ---- repulsion moment inversion -> per-row contribution ----
            # a,b = (s1 +- sqrt(2*s2 - s1^2))/2; d=sqrt(r2-v); contrib =
            # relu(0.02-da)+relu(0.02-db), gated by s1>0
            sh = [128, NB]
            t_t1 = scrp.tile(sh, f32, tag="rp1")
            t_t2 = scrp.tile(sh, f32, tag="rp2")
            t_sq = scrp.tile(sh, f32, tag="rp3")
            t_va = scrp.tile(sh, f32, tag="rp4")
            t_vb = scrp.tile(sh, f32, tag="rp5")
            t_ca = scrp.tile(sh, f32, tag="rp6")
            t_cb = scrp.tile(sh, f32, tag="rp7")
            t_msk = scrp.tile(sh, f32, tag="rp8")
            Alu_ = Alu
            nc.vector.tensor_tensor(t_t1[:], t_s1[:], t_s1[:], Alu_.mult)
            nc.vector.tensor_scalar(t_t2[:], t_s2[:], 2.0, None, Alu_.mult)
            nc.vector.tensor_tensor(t_t2[:], t_t2[:], t_t1[:], Alu_.subtract)
            nc.vector.tensor_scalar_max(t_t2[:], t_t2[:], 0.0)
            nc.scalar.activation(t_sq[:], t_t2[:], Act.Sqrt)
            nc.vector.tensor_tensor(t_va[:], t_s1[:], t_sq[:], Alu_.add)
            nc.vector.tensor_scalar(t_va[:], t_va[:], 0.5, R2, Alu_.mult, Alu_.min)
            nc.vector.tensor_tensor(t_vb[:], t_s1[:], t_sq[:], Alu_.subtract)
            nc.vector.tensor_scalar(t_vb[:], t_vb[:], 0.5, 0.0, Alu_.mult, Alu_.max)
            # da = sqrt(max(r2 - va, 1e-12)); contrib_a = max(0.02 - da, 0)
            for tv, tc_ in ((t_va, t_ca), (t_vb, t_cb)):
                nc.vector.tensor_scalar(tv[:], tv[:], -1.0, R2, Alu_.mult, Alu_.add)
                nc.vector.tensor_scalar_max(tv[:], tv[:], 1e-12)
                nc.scalar.activation(tv[:], tv[:], Act.Sqrt)
                nc.vector.tensor_scalar(tc_[:], tv[:], -1.0, float(REP_THRESH),
                                        Alu_.mult, Alu_.add)
                nc.vector.tensor_scalar_max(tc_[:], tc_[:], 0.0)
            nc.vector.tensor_scalar(t_msk[:], t_s1[:], 0.0, None, Alu_.is_gt)
            nc.vector.tensor_tensor(t_ca[:], t_ca[:], t_cb[:], Alu_.add)
            nc.vector.tensor_tensor(t_ca[:], t_ca[:], t_msk[:], Alu_.mult)
            t_reprow = persist.tile([128, 1], f32, tag="reprow")
            nc.vector.tensor_reduce(t_reprow[:], t_ca[:], Axis.X, Alu_.add)

            # ---- partition-sum [cd_row, rep] via DMA tree; pack scalars ----
            t_P2 = scrp.tile([128, 2], f32, tag="P2")
            t_P2s = scrp.tile([64, 2], f32, tag="P2s")
            nc.vector.tensor_copy(t_P2[:, 0:1], t_cdrow[:])
            nc.vector.tensor_copy(t_P2[:, 1:2], t_reprow[:])
            for h in [64, 32, 16, 8, 4, 2, 1]:
                nc.sync.dma_start(t_P2s[0:h, :], t_P2[h:2 * h, :])
                nc.vector.tensor_tensor(t_P2[0:h, :], t_P2[0:h, :],
                                        t_P2s[0:h, :], Alu.add)
            t_z6 = scrp.tile([6, 32], f16, tag="z6")
            nc.vector.memset(t_z6[:], 0.0)
            nc.vector.tensor_copy(t_z6[0:1, 0:2], t_P2[0:1, :])
            nc.vector.tensor_copy(t_z6[0:1, 2:3], t_cdcol[:])
            nc.sync.dma_start(out_g[:, N:N + 32], t_z6[:])

            normals_phase("g", out_g, do_rep=False)

    if split_waits:
        _split_excess_waits(nc, mybir)
    return nc


def _split_excess_waits(nc, mybir, max_w=1, max_u=1):
    """This toolchain's walrus accepts at most 1 sync wait and 1 update per
    instruction. Move excess waits onto same-engine prefix NoOps (the engine
    is in-order, so waiting earlier is equivalent) and excess updates onto
    suffix NoOps (signalling marginally later is safe)."""
    n = 0
    for func in nc.m.functions:
        for block in func.blocks:
            lst = block.instructions
            new = []
            for inst in lst:
                si = inst.sync_info
                ow = list(si.on_wait) if (si and si.on_wait) else []
                if len(ow) > max_w:
                    extra, keep = ow[:-max_w], ow[-max_w:]
                    for k in range(0, len(extra), max_w):
                        nop = mybir.InstNoOp(name=f"I-wsplit-{n}"); n += 1
                        nop.engine = inst.engine
                        nop.sync_info = mybir.SyncInfo(
                            on_wait=extra[k:k + max_w], on_update=[])
                        new.append(nop)
                    si.on_wait = keep
                new.append(inst)
                ou = list(si.on_update) if (si and si.on_update) else []
                if len(ou) > max_u:
                    keep_u, extra_u = ou[:max_u], ou[max_u:]
                    si.on_update = keep_u
                    for k in range(0, len(extra_u), max_u):
                        nop = mybir.InstNoOp(name=f"I-usplit-{n}"); n += 1
                        nop.engine = inst.engine
                        nop.sync_info = mybir.SyncInfo(
                            on_wait=[], on_update=extra_u[k:k + max_u])
                        new.append(nop)
            lst[:] = new
    return n


_NC_CACHE = None


def _get_nc():
    global _NC_CACHE
    if _NC_CACHE is None:
        _NC_CACHE = _build_nc()
    return _NC_CACHE


def _consts_np():
    negdiag = np.zeros((128, 128), dtype=BF16)
    np.fill_diagonal(negdiag, BF16(NEG_BIG))
    ident = np.zeros((128, 128), dtype=BF16)
    np.fill_diagonal(ident, BF16(1.0))
    return ident, negdiag


# ============================================================================
# Cached jit runner (replicates bass2jax.run_bass_via_pjrt, but the jitted
# executable, mesh, and const device buffers are built ONCE; the donated
# output buffer is recycled from the previous call's output)
# ============================================================================

class _Runner:
    def __init__(self):
        import jax
        from jax.sharding import Mesh, PartitionSpec, NamedSharding
        from jax.experimental.shard_map import shard_map
        from concourse import bass2jax
        import concourse.mybir as mybir

        self.jax = jax
        nc = _get_nc()
        bass2jax.install_neuronx_cc_hook()

        partition_name = (nc.partition_id_tensor.name
                          if nc.partition_id_tensor else None)
        in_names, out_names, out_avals, zero_outs = [], [], [], []
        for alloc in nc.m.functions[0].allocations:
            if not isinstance(alloc, mybir.MemoryLocationSet):
                continue
            name = alloc.memorylocations[0].name
            if alloc.kind == "ExternalInput":
                if name != partition_name:
                    in_names.append(name)
            elif alloc.kind == "ExternalOutput":
                shape = tuple(alloc.tensor_shape)
                dtype = mybir.dt.np(alloc.dtype)
                out_names.append(name)
                out_avals.append(jax.core.ShapedArray(shape, dtype))
                zero_outs.append((shape, dtype))
        assert in_names == ["pred", "gt", "ident", "negdiag"], in_names
        assert out_names == ["out_p", "out_g"], out_names
        n_params = len(in_names)
        n_outs = len(out_names)
        all_names = in_names + out_names
        if partition_name is not None:
            all_names.append(partition_name)
        self.zero_outs = zero_outs

        def _body(*args):
            operands = list(args)
            if partition_name is not None:
                operands.append(bass2jax.partition_id_tensor())
            outs = bass2jax._bass_exec_p.bind(
                *operands,
                out_avals=tuple(out_avals),
                in_names=tuple(all_names),
                out_names=tuple(out_names),
                lowering_input_output_aliases=(),
                sim_require_finite=True,
                sim_require_nnan=True,
                nc=nc,
            )
            return tuple(outs)

        devices = jax.devices()[:B]
        assert len(devices) == B, f"need {B} devices, have {len(jax.devices())}"
        mesh = Mesh(np.asarray(devices), ("core",))
        pspec = PartitionSpec("core")
        self._fn = jax.jit(
            shard_map(_body, mesh=mesh,
                      in_specs=(pspec,) * (n_params + n_outs),
                      out_specs=(pspec,) * n_outs,
                      check_rep=False),
            donate_argnums=tuple(range(n_params, n_params + n_outs)),
            keep_unused=True,
        )
        ident, negdiag = _consts_np()
        sh = NamedSharding(mesh, pspec)
        self._ident = jax.device_put(np.tile(ident, (B, 1)), sh)
        self._negdiag = jax.device_put(np.tile(negdiag, (B, 1)), sh)
        self._donate = None  # recycled output buffers
        self._fetch_pool = ThreadPoolExecutor(max_workers=2)

    def run(self, pred, gt):
        """pred, gt: [B, N, 3] f32 -> (fut_p, fut_g) resolving to host
        np.ndarrays [B*6, N] / [B*6, N+32] f16."""
        zeros = self._donate
        if zeros is None:
            zeros = [np.zeros((B * s[0],) + s[1:], d)
                     for s, d in self.zero_outs]
        out_p, out_g = self._fn(pred.reshape(B * N, DIM),
                                gt.reshape(B * N, DIM),
                                self._ident, self._negdiag, *zeros)
        # background fetches: the network wait releases the GIL, so the
        # host eigensolve for cloud p overlaps cloud g's transfer
        fut_p = self._fetch_pool.submit(np.asarray, out_p)
        fut_g = self._fetch_pool.submit(np.asarray, out_g)
        # the kernel writes every element of both outputs, so last call's
        # outputs can be donated as the next call's output buffers
        self._donate = [out_p, out_g]
        return fut_p, fut_g


_RUNNER = None


def _get_runner():
    global _RUNNER
    if _RUNNER is None:
        _RUNNER = _Runner()
    return _RUNNER


# ============================================================================
# Host combine
# ============================================================================

def _normals_from_covc(cv):
    """cv: [B, 6, N] centered covariance rows [xx,xy,xz,yy,yz,zz] (f16) ->
    [B*N, 3] smallest-eigval eigenvectors via np.linalg.eigh (ssyevd)."""
    f32 = np.float32
    cv = cv.astype(f32)
    cov = np.empty((cv.shape[0], cv.shape[2], 3, 3), dtype=f32)
    cov[:, :, 0, 0] = cv[:, 0]
    cov[:, :, 0, 1] = cov[:, :, 1, 0] = cv[:, 1]
    cov[:, :, 0, 2] = cov[:, :, 2, 0] = cv[:, 2]
    cov[:, :, 1, 1] = cv[:, 3]
    cov[:, :, 1, 2] = cov[:, :, 2, 1] = cv[:, 4]
    cov[:, :, 2, 2] = cv[:, 5]
    return np.linalg.eigh(cov.reshape(-1, 3, 3))[1][:, :, 0]


def _host_combine(fut_p, fut_g):
    """fut_p/fut_g: futures of device outputs [B*6, N] / [B*6, N+32] f16
    -> scalar loss f32."""
    arr_p = fut_p.result().reshape(B, 6, N)
    n_p = _normals_from_covc(arr_p)  # overlaps cloud-g transfer
    arr_g = fut_g.result().reshape(B, 6, N + 32)
    n_g = _normals_from_covc(arr_g[:, :, 0:N])
    dots = (n_p * n_g).sum(-1)
    normc = 1.0 - dots.mean(dtype=np.float64)

    scal = arr_g[:, 0, N:N + 3].astype(np.float64)
    cd = -(scal[:, 0].sum() + scal[:, 2].sum()) / (B * N)
    rep = scal[:, 1].sum() / (B * N * K_REP)

    return np.float32(CD_W * cd + REP_W * rep + NORM_W * normc)


# ============================================================================
# Entry point
# ============================================================================

def kernel(pred, gt):
    pred = np.ascontiguousarray(np.asarray(pred, dtype=np.float32))
    gt = np.ascontiguousarray(np.asarray(gt, dtype=np.float32))
    assert pred.shape == (B, N, DIM) and gt.shape == (B, N, DIM)
    fut_p, fut_g = _get_runner().run(pred, gt)
    return _host_combine(fut_p, fut_g)


if __name__ == "__main__":
    rng = np.random.default_rng(0)
    pred = rng.uniform(size=(B, N, DIM)).astype(np.float32)
    gt = rng.uniform(size=(B, N, DIM)).astype(np.float32)
    print("loss:", kernel(pred, gt))


# revision 18
# speedup vs baseline: 17.5930x; 1.9556x over previous
"""Trainium2 Bass kernel for nn_CombinedLoss (chamfer + repulsion + PCA-normal
consistency) on point clouds [8, 2048, 3].

Sharding: data-parallel over batch B=8 across 8 NeuronCores (1 sample/core).

v2 — restructured for the axon tunnel's ~80ms/RPC latency:
  - raw pred/gt uploaded (384KB total); ALL input prep happens on device
    (squared norms, fp32 distance-matmul operand rows, feature rows and
    their bf16 hi/lo transposed layout for the covariance matmul)
  - distance matrices -D via fp32 PE matmuls with K=5 augmented contraction
  - chamfer row/col reductions and the repulsion moment inversion are
    reduced to per-core SCALARS on device
  - everything is written to ONE fused output tensor [10, 4160] per core
    (cov moments of both clouds + 4 scalars) -> a single fetch round trip
  - the jitted shard_map executable is built once and cached; the donated
    output-zero buffer is recycled from the previous call's output
Host: centering + batched np.linalg.eigh (threaded) for the smallest-eigval
eigenvectors, then the weighted loss.
"""

import numpy as np
from concurrent.futures import ThreadPoolExecutor

try:
    import ml_dtypes

    BF16 = ml_dtypes.bfloat16
except Exception:  # pragma: no cover
    BF16 = None

B, N, DIM = 8, 2048, 3
K_REP = 4
REP_THRESH = np.float32(0.02)
R2 = float(np.float32(REP_THRESH) * np.float32(REP_THRESH))
K_NORM = 16
CD_W, REP_W, NORM_W = 1.0, 0.1, 0.01
NB = N // 128  # 16 row blocks
NEG_BIG = np.float32(-1e30)
SCAL_OFF = 2 * N          # scalar block starts here
OUT_COLS = 2 * N + 64     # fused output [10, OUT_COLS] f32 per core


# ============================================================================
# Bass device kernel builder
# ============================================================================

def _build_nc(split_waits=True):
    import concourse.bass as bass
    import concourse.mybir as mybir
    from concourse.tile import TileContext

    f32 = mybir.dt.float32
    bf16 = mybir.dt.bfloat16
    Alu = mybir.AluOpType
    Act = mybir.ActivationFunctionType
    Axis = mybir.AxisListType

    nc = bass.Bass()

    # ---- DRAM io (declaration order == jit operand order) ----
    f16 = mybir.dt.float16
    pred = nc.dram_tensor("pred", [N, DIM], f32, kind="ExternalInput")
    gt = nc.dram_tensor("gt", [N, DIM], f32, kind="ExternalInput")
    ident = nc.dram_tensor("ident", [128, 128], bf16, kind="ExternalInput")
    negdiag = nc.dram_tensor("negdiag", [128, 128], bf16, kind="ExternalInput")
    # centered covariance entries [xx,xy,xz,yy,yz,zz] per point, f16;
    # out_g additionally carries the 4 scalars in its last 32 columns
    out_p = nc.dram_tensor("out_p", [6, N], f16, kind="ExternalOutput")
    out_g = nc.dram_tensor("out_g", [6, N + 32], f16, kind="ExternalOutput")

    with TileContext(nc) as tc:
        import contextlib
        ctx = contextlib.ExitStack()
        with ctx:
            persist = ctx.enter_context(tc.tile_pool(name="persist", bufs=1))
            big = ctx.enter_context(tc.tile_pool(name="big", bufs=1))
            scrp = ctx.enter_context(tc.tile_pool(name="scr", bufs=1))
            ndmp = ctx.enter_context(tc.tile_pool(name="ndm", bufs=2))
            wtp = ctx.enter_context(tc.tile_pool(name="wtp", bufs=2))
            psd = ctx.enter_context(tc.tile_pool(name="psd", bufs=2, space="PSUM"))
            psc = ctx.enter_context(tc.tile_pool(name="psc", bufs=1, space="PSUM"))

            # ---- consts ----
            t_ident = persist.tile([128, 128], bf16, tag="ident")
            t_negdiag = persist.tile([128, 128], bf16, tag="ndg")
            nc.sync.dma_start(t_ident[:], ident[:])
            nc.sync.dma_start(t_negdiag[:], negdiag[:])
            t_ones = persist.tile([128, 128], bf16, tag="ones")
            nc.vector.memset(t_ones[:], 1.0)
            t_bias4 = persist.tile([128, 1], f32, tag="bias4")
            t_bias0 = persist.tile([128, 1], f32, tag="bias0")
            nc.vector.memset(t_bias4[:], R2)
            nc.vector.memset(t_bias0[:], 0.0)
            t_ones6 = persist.tile([1, 8], f32, tag="ones6")
            nc.vector.memset(t_ones6[:], 1.0)

            # ---- persistent per-cloud operand tiles ----
            A5 = {}; W5 = {}; F10 = {}; FT = {}
            for cl in ("p", "g"):
                A5[cl] = persist.tile([5, N], f32, tag=f"A5{cl}", name=f"A5{cl}")
                W5[cl] = persist.tile([5, N], f32, tag=f"W5{cl}", name=f"W5{cl}")
                F10[cl] = persist.tile([10, N], f32, tag=f"F10{cl}", name=f"F10{cl}")
                FT[cl] = persist.tile([128, NB * 20], bf16, tag=f"FT{cl}",
                                      name=f"FT{cl}")

            t_rowmax = persist.tile([128, NB, 2], f32, tag="rowmax")
            t_s1 = persist.tile([128, NB], f32, tag="s1")
            t_s2 = persist.tile([128, NB], f32, tag="s2")

            # ================= on-device prep =================
            # A5 = [2x, 2y, 2z, nn, 1] (fp32 matmul lhs rows)
            # W5 = [x, y, z, -1, -nn]  (fp32 matmul rhs rows)
            # F10 = [x2,xy,xz,y2,yz,z2,x,y,z,1] of centered coords
            # FT  = transposed bf16 hi/lo features [128, kb*20 + (0:10 hi|10:20 lo)]
            def prep(src_dram, cl):
                P3 = scrp.tile([3, N], f32, tag="P3")
                nc.sync.dma_start(P3[:], src_dram[:].rearrange("a b -> b a"))
                S3 = scrp.tile([3, N], f32, tag="S3")
                nc.vector.tensor_tensor(S3[:], P3[:], P3[:], Alu.mult)
                r1 = scrp.tile([1, N], f32, tag="r1")
                r2t = scrp.tile([1, N], f32, tag="r2t")
                nc.sync.dma_start(r1[:], S3[1:2, :])
                nc.sync.dma_start(r2t[:], S3[2:3, :])
                nn = scrp.tile([1, N], f32, tag="nn")
                nc.vector.tensor_tensor(nn[:], S3[0:1, :], r1[:], Alu.add)
                nc.vector.tensor_tensor(nn[:], nn[:], r2t[:], Alu.add)
                # engine ops may only start at partitions {0,32,64,96}: memset
                # the whole tile for the constant rows, DMA the odd-row writes
                a5, w5 = A5[cl], W5[cl]
                nc.vector.memset(a5[:], 1.0)
                nc.scalar.activation(a5[0:3, :], P3[:], Act.Copy, scale=2.0)
                nc.sync.dma_start(a5[3:4, :], nn[:])
                nc.vector.memset(w5[:], -1.0)
                nc.vector.tensor_copy(w5[0:3, :], P3[:])
                nnn = scrp.tile([1, N], f32, tag="nnn")
                nc.scalar.activation(nnn[:], nn[:], Act.Copy, scale=-1.0)
                nc.sync.dma_start(w5[4:5, :], nnn[:])
                # centered features
                C3 = scrp.tile([3, N], f32, tag="C3")
                nc.vector.tensor_scalar_add(C3[:], P3[:], -0.5)
                A6 = scrp.tile([6, N], f32, tag="A6")
                B6 = scrp.tile([6, N], f32, tag="B6")
                # A6 rows = [c0,c0,c0,c1,c1,c2]; B6 rows = [c0,c1,c2,c1,c2,c2]
                nc.vector.tensor_copy(A6[0:1, :], C3[0:1, :])
                nc.sync.dma_start(A6[1:2, :], C3[0:1, :])
                nc.sync.dma_start(A6[2:3, :], C3[0:1, :])
                nc.sync.dma_start(A6[3:4, :], C3[1:2, :])
                nc.sync.dma_start(A6[4:5, :], C3[1:2, :])
                nc.sync.dma_start(A6[5:6, :], C3[2:3, :])
                nc.vector.tensor_copy(B6[0:3, :], C3[:])
                nc.sync.dma_start(B6[3:5, :], C3[1:3, :])
                nc.sync.dma_start(B6[5:6, :], C3[2:3, :])
                f10 = F10[cl]
                nc.vector.memset(f10[:], 1.0)
                nc.vector.tensor_tensor(f10[0:6, :], A6[:], B6[:], Alu.mult)
                nc.sync.dma_start(f10[6:9, :], C3[:])
                # bf16 hi/lo split of features
                hi10 = scrp.tile([10, N], bf16, tag="hi10")
                hif = scrp.tile([10, N], f32, tag="hif")
                lo10f = scrp.tile([10, N], f32, tag="lo10f")
                lo10 = scrp.tile([10, N], bf16, tag="lo10")
                nc.scalar.activation(hi10[:], f10[:], Act.Copy)
                nc.scalar.activation(hif[:], hi10[:], Act.Copy)
                nc.vector.tensor_tensor(lo10f[:], f10[:], hif[:], Alu.subtract)
                nc.scalar.activation(lo10[:], lo10f[:], Act.Copy)
                # transpose [10, 128]-chunks -> FT[:, kb*20 + 0:10 / 10:20]
                ftt = FT[cl]
                for b in range(NB):
                    csl = slice(b * 128, (b + 1) * 128)
                    pst = psd.tile([128, 16], bf16, tag="dps")
                    nc.tensor.transpose(pst[:, 0:10], hi10[:, csl],
                                        t_ident[0:10, 0:10])
                    nc.scalar.activation(ftt[:, b * 20:b * 20 + 10], pst[:, 0:10],
                                         Act.Copy)
                    pst2 = psd.tile([128, 16], bf16, tag="dps")
                    nc.tensor.transpose(pst2[:, 0:10], lo10[:, csl],
                                        t_ident[0:10, 0:10])
                    nc.scalar.activation(ftt[:, b * 20 + 10:b * 20 + 20],
                                         pst2[:, 0:10], Act.Copy)

            prep(pred, "p")
            prep(gt, "g")

            # fp32 distance matmul: psum[128, 1024] = -D block (row block b,
            # column half h) between clouds (a5 lhs, w5 rhs)
            def build_half(a5, w5, b, h, ps):
                for j in range(2):
                    nc.tensor.matmul(
                        ps[:, j * 512:(j + 1) * 512],
                        a5[:, b * 128:(b + 1) * 128],
                        w5[:, h * 1024 + j * 512:h * 1024 + (j + 1) * 512],
                        start=True, stop=True,
                    )

            # ================= phase 1: chamfer on -Dpg =================
            t_colacc = big.tile([128, N], f32, tag="bigA")
            t_colred = big.tile([128, N], f32, tag="bigB")
            for b in range(NB):
                for h in range(2):
                    ps = psd.tile([128, 1024], f32, tag="dps")
                    build_half(A5["p"], W5["g"], b, h, ps)
                    nc.vector.tensor_reduce(t_rowmax[:, b, h:h + 1],
                                            ps[:], Axis.X, Alu.max)
                    cslice = slice(h * 1024, (h + 1) * 1024)
                    if b == 0:
                        nc.vector.tensor_copy(t_colacc[:, cslice], ps[:])
                    else:
                        nc.vector.tensor_tensor(t_colacc[:, cslice],
                                                t_colacc[:, cslice], ps[:], Alu.max)
            # partition-tree max 128 -> 1 (DMA crosses partitions, DVE cannot)
            for h in [64, 32, 16, 8, 4, 2, 1]:
                nc.sync.dma_start(t_colred[0:h, :], t_colacc[h:2 * h, :])
                nc.vector.tensor_tensor(t_colacc[0:h, :], t_colacc[0:h, :],
                                        t_colred[0:h, :], Alu.max)
            # chamfer scalars: sum of per-row maxes + sum of col maxes (of -D)
            t_cdcol = persist.tile([1, 1], f32, tag="cdcol")
            nc.vector.tensor_reduce(t_cdcol[:], t_colacc[0:1, :], Axis.X, Alu.add)
            t_rowfull = scrp.tile([128, NB], f32, tag="rowfull")
            nc.vector.tensor_reduce(t_rowfull[:], t_rowmax[:], Axis.X, Alu.max)
            t_cdrow = persist.tile([128, 1], f32, tag="cdrow")
            nc.vector.tensor_reduce(t_cdrow[:], t_rowfull[:], Axis.X, Alu.add)

            # ================= phases 2-4 for pp and gg =================
            def normals_phase(cl, out_dram, do_rep):
                a5, w5, ftt, f10 = A5[cl], W5[cl], FT[cl], F10[cl]

                def build_ndm(b):
                    ndm = ndmp.tile([128, N], bf16, tag="ndm", name=f"ndm{cl}{b}")
                    for h in range(2):
                        ps = psd.tile([128, 1024], f32, tag="dps")
                        build_half(a5, w5, b, h, ps)
                        nc.scalar.activation(ndm[:, h * 1024:(h + 1) * 1024],
                                             ps[:], Act.Copy)
                    nc.vector.tensor_tensor(
                        ndm[:, b * 128:(b + 1) * 128],
                        ndm[:, b * 128:(b + 1) * 128],
                        t_negdiag[:], Alu.add)
                    return ndm

                # pass 1: repulsion moments + 16-NN radius (tau) per row
                t_tau = scrp.tile([128, NB], f32, tag="tau")
                for b in range(NB):
                    ndm = build_ndm(b)
                    if do_rep:
                        scr = scrp.tile([128, N], bf16, tag="repscr")
                        scr2 = scrp.tile([128, N], bf16, tag="repscr2")
                        nc.scalar.activation(scr[:], ndm[:], Act.Relu,
                                             bias=t_bias4[:],
                                             accum_out=t_s1[:, b:b + 1])
                        nc.scalar.activation(scr2[:], scr[:], Act.Square,
                                             bias=t_bias0[:],
                                             accum_out=t_s2[:, b:b + 1])
                    t1 = scrp.tile([128, 1024], bf16, tag="tree1")
                    At = scrp.tile([128, 512], bf16, tag="treeA")
                    At2 = scrp.tile([128, 512], bf16, tag="treeA2")
                    m8a = scrp.tile([128, 8], bf16, tag="m8a")
                    m8b = scrp.tile([128, 8], bf16, tag="m8b")
                    nc.vector.tensor_tensor(t1[:], ndm[:, 0:1024],
                                            ndm[:, 1024:2048], Alu.max)
                    nc.vector.tensor_tensor(At[:], t1[:, 0:512],
                                            t1[:, 512:1024], Alu.max)
                    nc.vector.max(m8a[:], At[:])
                    nc.vector.match_replace(At2[:], m8a[:], At[:], float(NEG_BIG))
                    nc.vector.max(m8b[:], At2[:])
                    nc.vector.tensor_copy(t_tau[:, b:b + 1], m8b[:, 6:7])

                # tau broadcast: per-row tau -> [1, N] row -> PE ones-matmul
                # broadcast across partitions; mask compare is then direct on
                # the SYMMETRIC ndm blocks: wt[j, i] = (ndm[j, i] >= tau_i)
                t_taub = scrp.tile([128, 128], bf16, tag="taub")
                nc.vector.memset(t_taub[:], 0.0)
                nc.vector.tensor_copy(t_taub[:, 0:NB], t_tau[:])
                ps_tt = psd.tile([128, 128], bf16, tag="dps")
                nc.tensor.transpose(ps_tt[:], t_taub[:], t_ident[:])
                t_tt = scrp.tile([NB, 128], bf16, tag="tts")
                nc.scalar.activation(t_tt[:], ps_tt[0:NB, :], Act.Copy)
                t_tauT = scrp.tile([128, N], bf16, tag="tauT")
                nc.vector.memset(t_tauT[:], 0.0)
                nc.sync.dma_start(t_tauT[0:1, :], t_tt[:])
                t_taubc = scrp.tile([128, N], bf16, tag="taubc")
                for h in range(2):
                    ps_tau = psd.tile([128, 1024], f32, tag="dps")
                    for bb in range(8):
                        c0 = h * 1024 + bb * 128
                        nc.tensor.matmul(ps_tau[:, bb * 128:(bb + 1) * 128],
                                         t_ones[:],
                                         t_tauT[:, c0:c0 + 128],
                                         start=True, stop=True)
                    nc.scalar.activation(t_taubc[:, h * 1024:(h + 1) * 1024],
                                         ps_tau[:], Act.Copy)

                # pass 2: rebuild -D per block, mask, accumulate covariance
                # moments cps[10, N] over kb (hi+lo)
                cps = psc.tile([10, N], f32, tag="cps")
                for kb in range(NB):
                    ndm = build_ndm(kb)
                    wt = wtp.tile([128, N], bf16, tag="wt", name=f"wt{cl}{kb}")
                    nc.vector.tensor_tensor(wt[:], ndm[:], t_taubc[:], Alu.is_ge)
                    for j in range(4):
                        cols = slice(j * 512, (j + 1) * 512)
                        for half in range(2):
                            nc.tensor.matmul(
                                cps[:, cols],
                                ftt[:, kb * 20 + half * 10:kb * 20 + (half + 1) * 10],
                                wt[:, cols],
                                start=(kb == 0 and half == 0),
                                stop=(kb == NB - 1 and half == 1))
                # self add, then center on device:
                #   covc[ab] = M2[ab]/cnt - (s[a]/cnt)*(s[b]/cnt)   (f16 out)
                covsb = big.tile([10, N], f32, tag="bigA", name=f"covsb{cl}")
                nc.vector.tensor_tensor(covsb[:], cps[:], f10[:], Alu.add)
                rr = scrp.tile([1, N], f32, tag="r1")
                nc.sync.dma_start(rr[:], covsb[9:10, :])
                rcp = scrp.tile([1, N], f32, tag="r2t")
                nc.vector.reciprocal(rcp[:], rr[:])
                mus = scrp.tile([3, N], f32, tag="S3")
                nc.sync.dma_start(mus[:], covsb[6:9, :])
                psB3 = psc.tile([3, N], f32, tag="cps", name=f"psB3{cl}")
                for j in range(4):
                    cj = slice(j * 512, (j + 1) * 512)
                    nc.tensor.matmul(psB3[:, cj], t_ones6[0:1, 0:3], rcp[:, cj],
                                     start=True, stop=True)
                mu3 = scrp.tile([3, N], f32, tag="C3")
                nc.vector.tensor_tensor(mu3[:], mus[:], psB3[:], Alu.mult)
                A6m = scrp.tile([6, N], f32, tag="A6")
                B6m = scrp.tile([6, N], f32, tag="B6")
                nc.vector.tensor_copy(A6m[0:1, :], mu3[0:1, :])
                nc.sync.dma_start(A6m[1:2, :], mu3[0:1, :])
                nc.sync.dma_start(A6m[2:3, :], mu3[0:1, :])
                nc.sync.dma_start(A6m[3:4, :], mu3[1:2, :])
                nc.sync.dma_start(A6m[4:5, :], mu3[1:2, :])
                nc.sync.dma_start(A6m[5:6, :], mu3[2:3, :])
                nc.vector.tensor_copy(B6m[0:3, :], mu3[:])
                nc.sync.dma_start(B6m[3:5, :], mu3[1:3, :])
                nc.sync.dma_start(B6m[5:6, :], mu3[2:3, :])
                P6 = scrp.tile([6, N], f32, tag="lo10f")
                nc.vector.tensor_tensor(P6[:], A6m[:], B6m[:], Alu.mult)
                psB6 = psc.tile([6, N], f32, tag="cps", name=f"psB6{cl}")
                for j in range(4):
                    cj = slice(j * 512, (j + 1) * 512)
                    nc.tensor.matmul(psB6[:, cj], t_ones6[0:1, 0:6], rcp[:, cj],
                                     start=True, stop=True)
                M2r = scrp.tile([6, N], f32, tag="hif")
                nc.vector.tensor_tensor(M2r[:], covsb[0:6, :], psB6[:], Alu.mult)
                covc = scrp.tile([6, N], f16, tag="hi10")
                nc.vector.tensor_tensor(covc[:], M2r[:], P6[:], Alu.subtract)
                nc.sync.dma_start(out_dram[:, 0:N], covc[:])

            normals_phase("p", out_p, do_rep=True)

            # ---- repulsion moment inversion -> per-row contribution ----
            # a,b = (s1 +- sqrt(2*s2 - s1^2))/2; d=sqrt(r2-v); contrib =
            # relu(0.02-da)+relu(0.02-db), gated by s1>0
            sh = [128, NB]
            t_t1 = scrp.tile(sh, f32, tag="rp1")
            t_t2 = scrp.tile(sh, f32, tag="rp2")
            t_sq = scrp.tile(sh, f32, tag="rp3")
            t_va = scrp.tile(sh, f32, tag="rp4")
            t_vb = scrp.tile(sh, f32, tag="rp5")
            t_ca = scrp.tile(sh, f32, tag="rp6")
            t_cb = scrp.tile(sh, f32, tag="rp7")
            t_msk = scrp.tile(sh, f32, tag="rp8")
            Alu_ = Alu
            nc.vector.tensor_tensor(t_t1[:], t_s1[:], t_s1[:], Alu_.mult)
            nc.vector.tensor_scalar(t_t2[:], t_s2[:], 2.0, None, Alu_.mult)
            nc.vector.tensor_tensor(t_t2[:], t_t2[:], t_t1[:], Alu_.subtract)
            nc.vector.tensor_scalar_max(t_t2[:], t_t2[:], 0.0)
            nc.scalar.activation(t_sq[:], t_t2[:], Act.Sqrt)
            nc.vector.tensor_tensor(t_va[:], t_s1[:], t_sq[:], Alu_.add)
            nc.vector.tensor_scalar(t_va[:], t_va[:], 0.5, R2, Alu_.mult, Alu_.min)
            nc.vector.tensor_tensor(t_vb[:], t_s1[:], t_sq[:], Alu_.subtract)
            nc.vector.tensor_scalar(t_vb[:], t_vb[:], 0.5, 0.0, Alu_.mult, Alu_.max)
            # da = sqrt(max(r2 - va, 1e-12)); contrib_a = max(0.02 - da, 0)
            for tv, tc_ in ((t_va, t_ca), (t_vb, t_cb)):
                nc.vector.tensor_scalar(tv[:], tv[:], -1.0, R2, Alu_.mult, Alu_.add)
                nc.vector.tensor_scalar_max(tv[:], tv[:], 1e-12)
                nc.scalar.activation(tv[:], tv[:], Act.Sqrt)
                nc.vector.tensor_scalar(tc_[:], tv[:], -1.0, float(REP_THRESH),
                                        Alu_.mult, Alu_.add)
                nc.vector.tensor_scalar_max(tc_[:], tc_[:], 0.0)
            nc.vector.tensor_scalar(t_msk[:], t_s1[:], 0.0, None, Alu_.is_gt)
            nc.vector.tensor_tensor(t_ca[:], t_ca[:], t_cb[:], Alu_.add)
            nc.vector.tensor_tensor(t_ca[:], t_ca[:], t_msk[:], Alu_.mult)
            t_reprow = persist.tile([128, 1], f32, tag="reprow")
            nc.vector.tensor_reduce(t_reprow[:], t_ca[:], Axis.X, Alu_.add)

            # ---- partition-sum [cd_row, rep] via DMA tree; pack scalars ----
            t_P2 = scrp.tile([128, 2], f32, tag="P2")
            t_P2s = scrp.tile([64, 2], f32, tag="P2s")
            nc.vector.tensor_copy(t_P2[:, 0:1], t_cdrow[:])
            nc.vector.tensor_copy(t_P2[:, 1:2], t_reprow[:])
            for h in [64, 32, 16, 8, 4, 2, 1]:
                nc.sync.dma_start(t_P2s[0:h, :], t_P2[h:2 * h, :])
                nc.vector.tensor_tensor(t_P2[0:h, :], t_P2[0:h, :],
                                        t_P2s[0:h, :], Alu.add)
            t_z6 = scrp.tile([6, 32], f16, tag="z6")
            nc.vector.memset(t_z6[:], 0.0)
            nc.vector.tensor_copy(t_z6[0:1, 0:2], t_P2[0:1, :])
            nc.vector.tensor_copy(t_z6[0:1, 2:3], t_cdcol[:])
            nc.sync.dma_start(out_g[:, N:N + 32], t_z6[:])

            normals_phase("g", out_g, do_rep=False)

    if split_waits:
        _split_excess_waits(nc, mybir)
    return nc


def _split_excess_waits(nc, mybir, max_w=1, max_u=1):
    """This toolchain's walrus accepts at most 1 sync wait and 1 update per
    instruction. Move excess waits onto same-engine prefix NoOps (the engine
    is in-order, so waiting earlier is equivalent) and excess updates onto
    suffix NoOps (signalling marginally later is safe)."""
    n = 0
    for func in nc.m.functions:
        for block in func.blocks:
            lst = block.instructions
            new = []
            for inst in lst:
                si = inst.sync_info
                ow = list(si.on_wait) if (si and si.on_wait) else []
                if len(ow) > max_w:
                    extra, keep = ow[:-max_w], ow[-max_w:]
                    for k in range(0, len(extra), max_w):
                        nop = mybir.InstNoOp(name=f"I-wsplit-{n}"); n += 1
                        nop.engine = inst.engine
                        nop.sync_info = mybir.SyncInfo(
                            on_wait=extra[k:k + max_w], on_update=[])
                        new.append(nop)
                    si.on_wait = keep
                new.append(inst)
                ou = list(si.on_update) if (si and si.on_update) else []
                if len(ou) > max_u:
                    keep_u, extra_u = ou[:max_u], ou[max_u:]
                    si.on_update = keep_u
                    for k in range(0, len(extra_u), max_u):
                        nop = mybir.InstNoOp(name=f"I-usplit-{n}"); n += 1
                        nop.engine = inst.engine
                        nop.sync_info = mybir.SyncInfo(
                            on_wait=[], on_update=extra_u[k:k + max_u])
                        new.append(nop)
            lst[:] = new
    return n


_NC_CACHE = None


def _get_nc():
    global _NC_CACHE
    if _NC_CACHE is None:
        _NC_CACHE = _build_nc()
    return _NC_CACHE


def _consts_np():
    negdiag = np.zeros((128, 128), dtype=BF16)
    np.fill_diagonal(negdiag, BF16(NEG_BIG))
    ident = np.zeros((128, 128), dtype=BF16)
    np.fill_diagonal(ident, BF16(1.0))
    return ident, negdiag


# ============================================================================
# Cached jit runner (replicates bass2jax.run_bass_via_pjrt, but the jitted
# executable, mesh, and const device buffers are built ONCE; the donated
# output buffer is recycled from the previous call's output)
# ============================================================================

class _Runner:
    def __init__(self):
        import jax
        from jax.sharding import Mesh, PartitionSpec, NamedSharding
        from jax.experimental.shard_map import shard_map
        from concourse import bass2jax
        import concourse.mybir as mybir

        self.jax = jax
        nc = _get_nc()
        bass2jax.install_neuronx_cc_hook()

        partition_name = (nc.partition_id_tensor.name
                          if nc.partition_id_tensor else None)
        in_names, out_names, out_avals, zero_outs = [], [], [], []
        for alloc in nc.m.functions[0].allocations:
            if not isinstance(alloc, mybir.MemoryLocationSet):
                continue
            name = alloc.memorylocations[0].name
            if alloc.kind == "ExternalInput":
                if name != partition_name:
                    in_names.append(name)
            elif alloc.kind == "ExternalOutput":
                shape = tuple(alloc.tensor_shape)
                dtype = mybir.dt.np(alloc.dtype)
                out_names.append(name)
                out_avals.append(jax.core.ShapedArray(shape, dtype))
                zero_outs.append((shape, dtype))
        assert in_names == ["pred", "gt", "ident", "negdiag"], in_names
        assert out_names == ["out_p", "out_g"], out_names
        n_params = len(in_names)
        n_outs = len(out_names)
        all_names = in_names + out_names
        if partition_name is not None:
            all_names.append(partition_name)
        self.zero_outs = zero_outs

        def _body(*args):
            operands = list(args)
            if partition_name is not None:
                operands.append(bass2jax.partition_id_tensor())
            outs = bass2jax._bass_exec_p.bind(
                *operands,
                out_avals=tuple(out_avals),
                in_names=tuple(all_names),
                out_names=tuple(out_names),
                lowering_input_output_aliases=(),
                sim_require_finite=True,
                sim_require_nnan=True,
                nc=nc,
            )
            return tuple(outs)

        devices = jax.devices()[:B]
        assert len(devices) == B, f"need {B} devices, have {len(jax.devices())}"
        mesh = Mesh(np.asarray(devices), ("core",))
        pspec = PartitionSpec("core")
        self._fn = jax.jit(
            shard_map(_body, mesh=mesh,
                      in_specs=(pspec,) * (n_params + n_outs),
                      out_specs=(pspec,) * n_outs,
                      check_rep=False),
            donate_argnums=tuple(range(n_params, n_params + n_outs)),
            keep_unused=True,
        )
        ident, negdiag = _consts_np()
        sh = NamedSharding(mesh, pspec)
        self._ident = jax.device_put(np.tile(ident, (B, 1)), sh)
        self._negdiag = jax.device_put(np.tile(negdiag, (B, 1)), sh)
        self._donate = None  # recycled output buffers
        self._fetch_pool = ThreadPoolExecutor(max_workers=2)

    def run(self, pred, gt):
        """pred, gt: [B, N, 3] f32 -> (fut_p, fut_g) resolving to host
        np.ndarrays [B*6, N] / [B*6, N+32] f16."""
        zeros = self._donate
        if zeros is None:
            zeros = [np.zeros((B * s[0],) + s[1:], d)
                     for s, d in self.zero_outs]
        out_p, out_g = self._fn(pred.reshape(B * N, DIM),
                                gt.reshape(B * N, DIM),
                                self._ident, self._negdiag, *zeros)
        # background fetches: the network wait releases the GIL, so the
        # host eigensolve for cloud p overlaps cloud g's transfer
        fut_p = self._fetch_pool.submit(np.asarray, out_p)
        fut_g = self._fetch_pool.submit(np.asarray, out_g)
        # the kernel writes every element of both outputs, so last call's
        # outputs can be donated as the next call's output buffers
        self._donate = [out_p, out_g]
        return fut_p, fut_g


_RUNNER = None


def _get_runner():
    global _RUNNER
    if _RUNNER is None:
        _RUNNER = _Runner()
    return _RUNNER


# ============================================================================
# Host combine
# ============================================================================

# ----------------------------------------------------------------------------
# LAPACK ssyevd 3x3 sign-convention replication (fp32), numba scalar port of
# the vectorized replica validated 100% against jax/scipy CPU eigh signs.
# Falls back to np.linalg.eigh (99.35% sign agreement) without numba.
# ----------------------------------------------------------------------------
try:
    from numba import njit as _njit
    _HAVE_NUMBA = True
except Exception:  # pragma: no cover
    _HAVE_NUMBA = False

if _HAVE_NUMBA:
    _F = np.float32
    _EPS = _F(2.0) ** _F(-24)
    _EPS2 = _F(_EPS * _EPS)
    _SAFMIN = _F(1.1754943508222875e-38)
    _ONE = _F(1.0)
    _TWO = _F(2.0)
    _HALF = _F(0.5)
    _ZERO = _F(0.0)

    @_njit(cache=True, fastmath=False)
    def _fsign(a, b):
        return np.abs(a) if b >= _ZERO else -np.abs(a)

    @_njit(cache=True, fastmath=False)
    def _slapy2(x, y):
        ax = np.abs(x); ay = np.abs(y)
        w = max(ax, ay); z = min(ax, ay)
        if z == _ZERO:
            return w
        r = z / w
        return w * np.sqrt(_ONE + r * r)

    @_njit(cache=True, fastmath=False)
    def _slartg(f, g):
        if g == _ZERO:
            return _ONE, _ZERO, f
        if f == _ZERO:
            return _ZERO, _fsign(_ONE, g), np.abs(g)
        d = np.sqrt(f * f + g * g)
        cs = np.abs(f) / d
        r = _fsign(d, f)
        sn = g / r
        return cs, sn, r

    @_njit(cache=True, fastmath=False)
    def _slaev2(a, b, c):
        sm = a + c
        df = a - c
        adf = np.abs(df)
        tb = b + b
        ab_ = np.abs(tb)
        if np.abs(a) > np.abs(c):
            acmx = a; acmn = c
        else:
            acmx = c; acmn = a
        if adf > ab_:
            r_ = ab_ / adf
            rt = adf * np.sqrt(_ONE + r_ * r_)
        elif adf < ab_:
            r_ = adf / ab_
            rt = ab_ * np.sqrt(_ONE + r_ * r_)
        else:
            rt = ab_ * np.sqrt(_TWO)
        if sm < _ZERO:
            rt1 = _HALF * (sm - rt)
            sgn1 = -_ONE
            rt2 = (acmx / rt1) * acmn - (b / rt1) * b
        elif sm > _ZERO:
            rt1 = _HALF * (sm + rt)
            sgn1 = _ONE
            rt2 = (acmx / rt1) * acmn - (b / rt1) * b
        else:
            rt1 = _HALF * rt
            sgn1 = _ONE
            rt2 = -_HALF * rt
        if df >= _ZERO:
            cs = df + rt
            sgn2 = _ONE
        else:
            cs = df - rt
            sgn2 = -_ONE
        acs = np.abs(cs)
        if acs > ab_:
            ct = -tb / cs
            sn1 = _ONE / np.sqrt(_ONE + ct * ct)
            cs1 = ct * sn1
        else:
            if ab_ == _ZERO:
                cs1 = _ONE
                sn1 = _ZERO
            else:
                tn = -cs / tb
                cs1 = _ONE / np.sqrt(_ONE + tn * tn)
                sn1 = tn * cs1
        if sgn1 == sgn2:
            t = cs1
            cs1 = -sn1
            sn1 = t
        return rt1, rt2, cs1, sn1

    @_njit(cache=True, fastmath=False)
    def _rot(Z, ca, cb, c, s):
        for i in range(3):
            temp = Z[i, cb]
            Z[i, cb] = c * temp - s * Z[i, ca]
            Z[i, ca] = s * temp + c * Z[i, ca]

    @_njit(cache=True, fastmath=False)
    def _eigh3_batch(cv, out):
        # cv: [M, 6] f32 (xx, xy, xz, yy, yz, zz); out: [M, 3]
        Z = np.empty((3, 3), np.float32)
        for idx in range(cv.shape[0]):
            a00 = cv[idx, 0]; a10 = cv[idx, 1]; a20 = cv[idx, 2]
            a11 = cv[idx, 3]; a21 = cv[idx, 4]; a22 = cv[idx, 5]
            # ssytd2 lower
            xnorm = np.abs(a20)
            alpha = a10
            beta = -_fsign(_slapy2(alpha, xnorm), alpha)
            refl = xnorm != _ZERO
            if refl:
                tau1 = (beta - alpha) / beta
                v2 = a20 / (alpha - beta)
                w1 = tau1 * a11 + tau1 * (a21 * v2)
                w2 = tau1 * a21 + (tau1 * v2) * a22
                alp = -_HALF * tau1 * (w1 + w2 * v2)
                w1 = w1 + alp
                w2 = w2 + alp * v2
                d0 = a00
                d1 = a11 - (w1 + w1)
                d2 = a22 - ((v2 * w2) + (v2 * w2))
                e0 = beta
                e1 = a21 - (v2 * w1 + w2)
            else:
                tau1 = _ZERO
                v2 = _ZERO
                d0 = a00; d1 = a11; d2 = a22
                e0 = a10; e1 = a21
            for i in range(3):
                for j in range(3):
                    Z[i, j] = _ONE if i == j else _ZERO
            s0 = np.abs(e0) <= (np.sqrt(np.abs(d0)) * np.sqrt(np.abs(d1))) * _EPS
            s1m = np.abs(e1) <= (np.sqrt(np.abs(d1)) * np.sqrt(np.abs(d2))) * _EPS
            if s0:
                e0 = _ZERO
            if s1m:
                e1 = _ZERO
            if s0 and not s1m:
                tst = e1 * e1
                thr = (_EPS2 * np.abs(d1)) * np.abs(d2) + _SAFMIN
                if tst > thr:
                    rt1, rt2, c, s = _slaev2(d1, e1, d2)
                    _rot(Z, 1, 2, c, s)
                    d1 = rt1; d2 = rt2
                e1 = _ZERO
            elif (not s0) and s1m:
                tst = e0 * e0
                thr = (_EPS2 * np.abs(d0)) * np.abs(d1) + _SAFMIN
                if tst > thr:
                    rt1, rt2, c, s = _slaev2(d0, e0, d1)
                    _rot(Z, 0, 1, c, s)
                    d0 = rt1; d1 = rt2
                e0 = _ZERO
            elif (not s0) and (not s1m):
                if np.abs(d2) < np.abs(d0):
                    # QR variant
                    l = 2
                    for _it in range(40):
                        if l <= -1:
                            break
                        if l == 2:
                            m2s = e1 * e1 <= (_EPS2 * np.abs(d2)) * np.abs(d1) + _SAFMIN
                            m1s = e0 * e0 <= (_EPS2 * np.abs(d1)) * np.abs(d0) + _SAFMIN
                            if m2s:
                                e1 = _ZERO
                                l = 1
                            elif m1s:
                                e0 = _ZERO
                                rt1, rt2, c, s = _slaev2(d1, e1, d2)
                                _rot(Z, 1, 2, c, s)
                                d1 = rt1; d2 = rt2
                                e1 = _ZERO
                                l = 0
                            else:
                                P = d2
                                G = (d1 - P) / (_TWO * e1)
                                R = _slapy2(G, _ONE)
                                G = d0 - P + (e1 / (G + _fsign(R, G)))
                                Fv = e0
                                Bv = e0
                                C, S, R = _slartg(G, Fv)
                                G2 = d0
                                R = (d1 - G2) * S + (_TWO * C) * Bv
                                Pv = S * R
                                d0n = G2 + Pv
                                G = C * R - Bv
                                c0 = C; s0_ = S
                                Fv = S * e1
                                Bv = C * e1
                                C, S, R = _slartg(G, Fv)
                                e0n = R
                                G2 = d1 - Pv
                                R = (d2 - G2) * S + (_TWO * C) * Bv
                                Pv2 = S * R
                                d1n = G2 + Pv2
                                G = C * R - Bv
                                c1 = C; s1_ = S
                                _rot(Z, 0, 1, c0, s0_)
                                _rot(Z, 1, 2, c1, s1_)
                                d0 = d0n; d1 = d1n; d2 = d2 - Pv2
                                e0 = e0n; e1 = G
                        elif l == 1:
                            ms = e0 * e0 <= (_EPS2 * np.abs(d1)) * np.abs(d0) + _SAFMIN
                            if ms:
                                e0 = _ZERO
                                l = 0
                            else:
                                rt1, rt2, c, s = _slaev2(d0, e0, d1)
                                _rot(Z, 0, 1, c, s)
                                d0 = rt1; d1 = rt2
                                e0 = _ZERO
                                l = -1
                        else:  # l == 0
                            l = -1
                else:
                    # QL variant
                    l = 0
                    for _it in range(40):
                        if l >= 3:
                            break
                        if l == 0:
                            m0s = e0 * e0 <= (_EPS2 * np.abs(d0)) * np.abs(d1) + _SAFMIN
                            m1s = e1 * e1 <= (_EPS2 * np.abs(d1)) * np.abs(d2) + _SAFMIN
                            if m0s:
                                e0 = _ZERO
                                l = 1
                            elif m1s:
                                e1 = _ZERO
                                rt1, rt2, c, s = _slaev2(d0, e0, d1)
                                _rot(Z, 0, 1, c, s)
                                d0 = rt1; d1 = rt2
                                e0 = _ZERO
                                l = 2
                            else:
                                P = d0
                                G = (d1 - P) / (_TWO * e0)
                                R = _slapy2(G, _ONE)
                                G = d2 - P + (e0 / (G + _fsign(R, G)))
                                Fv = e1
                                Bv = e1
                                C, S, R = _slartg(G, Fv)
                                G2 = d2
                                R = (d1 - G2) * S + (_TWO * C) * Bv
                                Pv = S * R
                                d2n = G2 + Pv
                                G = C * R - Bv
                                c1 = C; s1_ = -S
                                Fv = S * e0
                                Bv = C * e0
                                C, S, R = _slartg(G, Fv)
                                e1n = R
                                G2 = d1 - Pv
                                R = (d0 - G2) * S + (_TWO * C) * Bv
                                Pv2 = S * R
                                d1n = G2 + Pv2
                                G = C * R - Bv
                                c0 = C; s0_ = -S
                                _rot(Z, 1, 2, c1, s1_)
                                _rot(Z, 0, 1, c0, s0_)
                                d2 = d2n; d1 = d1n; d0 = d0 - Pv2
                                e1 = e1n; e0 = G
                        elif l == 1:
                            ms = e1 * e1 <= (_EPS2 * np.abs(d1)) * np.abs(d2) + _SAFMIN
                            if ms:
                                e1 = _ZERO
                                l = 2
                            else:
                                rt1, rt2, c, s = _slaev2(d1, e1, d2)
                                _rot(Z, 1, 2, c, s)
                                d1 = rt1; d2 = rt2
                                e1 = _ZERO
                                l = 3
                        else:  # l == 2
                            l = 3
            # sort eigenvalues ascending, swapping Z columns (ssteqr tail)
            D0 = d0; D1 = d1; D2 = d2
            for i in range(2):
                if i == 0:
                    k = 0; P = D0
                    if D1 < P:
                        k = 1; P = D1
                    if D2 < P:
                        k = 2; P = D2
                    if k != 0:
                        if k == 1:
                            D1 = D0
                        else:
                            D2 = D0
                        D0 = P
                        for r_i in range(3):
                            t = Z[r_i, 0]; Z[r_i, 0] = Z[r_i, k]; Z[r_i, k] = t
                else:
                    if D2 < D1:
                        t2 = D1; D1 = D2; D2 = t2
                        for r_i in range(3):
                            t = Z[r_i, 1]; Z[r_i, 1] = Z[r_i, 2]; Z[r_i, 2] = t
            # back-transform the householder (sorm2r)
            if refl:
                for col in range(3):
                    w = Z[1, col] + v2 * Z[2, col]
                    Z[1, col] = Z[1, col] - tau1 * w
                    Z[2, col] = Z[2, col] - (tau1 * v2) * w
            out[idx, 0] = Z[0, 0]
            out[idx, 1] = Z[1, 0]
            out[idx, 2] = Z[2, 0]


def _normals_from_covc(cv):
    """cv: [B, 6, N] centered covariance rows [xx,xy,xz,yy,yz,zz] (f16) ->
    [B*N, 3] smallest-eigval eigenvectors with ssyevd sign convention."""
    f32 = np.float32
    flat = np.ascontiguousarray(
        cv.astype(f32).transpose(0, 2, 1).reshape(-1, 6))
    if _HAVE_NUMBA:
        out = np.empty((flat.shape[0], 3), f32)
        _eigh3_batch(flat, out)
        return out
    cov = np.empty((flat.shape[0], 3, 3), dtype=f32)
    cov[:, 0, 0] = flat[:, 0]
    cov[:, 0, 1] = cov[:, 1, 0] = flat[:, 1]
    cov[:, 0, 2] = cov[:, 2, 0] = flat[:, 2]
    cov[:, 1, 1] = flat[:, 3]
    cov[:, 1, 2] = cov[:, 2, 1] = flat[:, 4]
    cov[:, 2, 2] = flat[:, 5]
    return np.linalg.eigh(cov)[1][:, :, 0]


def _host_combine(fut_p, fut_g):
    """fut_p/fut_g: futures of device outputs [B*6, N] / [B*6, N+32] f16
    -> scalar loss f32."""
    arr_p = fut_p.result().reshape(B, 6, N)
    n_p = _normals_from_covc(arr_p)  # overlaps cloud-g transfer
    arr_g = fut_g.result().reshape(B, 6, N + 32)
    n_g = _normals_from_covc(arr_g[:, :, 0:N])
    dots = (n_p * n_g).sum(-1)
    normc = 1.0 - dots.mean(dtype=np.float64)

    scal = arr_g[:, 0, N:N + 3].astype(np.float64)
    cd = -(scal[:, 0].sum() + scal[:, 2].sum()) / (B * N)
    rep = scal[:, 1].sum() / (B * N * K_REP)

    return np.float32(CD_W * cd + REP_W * rep + NORM_W * normc)


# ============================================================================
# Entry point
# ============================================================================

def kernel(pred, gt):
    pred = np.ascontiguousarray(np.asarray(pred, dtype=np.float32))
    gt = np.ascontiguousarray(np.asarray(gt, dtype=np.float32))
    assert pred.shape == (B, N, DIM) and gt.shape == (B, N, DIM)
    fut_p, fut_g = _get_runner().run(pred, gt)
    return _host_combine(fut_p, fut_g)


if __name__ == "__main__":
    rng = np.random.default_rng(0)
    pred = rng.uniform(size=(B, N, DIM)).astype(np.float32)
    gt = rng.uniform(size=(B, N, DIM)).astype(np.float32)
    print("loss:", kernel(pred, gt))
